# revision 28
# baseline (speedup 1.0000x reference)
"""Trainium2 Bass kernel for a GPT-style transformer block.

Problem: nn_Block_36807869727037 (dense_transformer)
  B=4, T=2048, C=1024, H=16 heads (d=64), fp32 I/O.
  y = x + attn(LN1(x)); y = y + mlp(LN2(y))  (causal attention, tanh-GELU MLP)

Sharding (8 cores, one uniform SPMD program):
  core = 2*b + s  -> batch b in [0,4), tensor-parallel shard s in [0,2).
  Shard s owns heads [8s, 8s+8) and FFN hidden slice [2048s, 2048s+2048).
  Each core runs the full sequence (T=2048) for its batch:
    LN1 (duplicated in pair) -> QKV for its 8 heads -> causal attention ->
    c_proj partial (+b_proj/2) -> pairwise AllReduce (bf16) -> x2 = x + cc ->
    LN2 (duplicated) -> fc half + GELU -> mlp_proj partial.
  Final combine on host: out[b] = x2 (from core 2b) + mlp_partial(2b)
                                  + mlp_partial(2b+1) + b_mlp_proj.

Precision: QKV projection, scores (QK^T) and PV run in fp8e4 with
DoubleRow perf mode (2x PE throughput); c_proj and the MLP stay bf16.
PSUM accumulation is fp32 everywhere. Scores are computed transposed
S^T[k,q]; softmax uses no max-subtraction (|scores/8| < ~3); the causal
mask is applied additively on PSUM scores BEFORE exp; the denominator
comes from an appended ones-column in V and is applied via DVE
reciprocal + GpSimd partition-broadcast (no tensor-engine involvement).
LayerNorm rstd uses a DVE-only Newton rsqrt so the scalar engine keeps
the exp activation table resident for the whole attention phase.
"""

import sys

sys.path.insert(0, "/opt/trn_rl_repo")

import numpy as np
import ml_dtypes

B, T, C, H = 4, 2048, 1024, 16
D = C // H          # 64 head dim
HPC = H // 2        # 8 heads per core
FPC = 2 * C         # 2048 ffn hidden per core
P = 128
NT = T // P         # 16 token tiles
NR = T // 512       # 4 query runs of 512
CK = C // P         # 8 feature chunks (bf16 path)
CKD = C // 256      # 4 double-row feature chunks (fp8 path)
FK = FPC // P       # 16 ffn chunks per core
EK = (HPC * D) // P  # 4 head-dim chunks per core (512/128)
MAGIC = 0x5F3759DF

_CACHED = {}


def _build_nc():
    import concourse.bass as bass
    import concourse.mybir as mybir
    import concourse.tile as tile
    from concourse import bacc

    f32 = mybir.dt.float32
    bf16 = mybir.dt.bfloat16
    fp8 = mybir.dt.float8e4
    u16 = mybir.dt.uint16
    i32 = mybir.dt.int32
    AF = mybir.ActivationFunctionType
    ALU = mybir.AluOpType
    DR = mybir.MatmulPerfMode.DoubleRow

    nc = bacc.Bacc(trn_type="TRN2", target_bir_lowering=False, num_devices=8)

    # ---- I/O ----
    x_d = nc.dram_tensor("x", [T, C], f32, kind="ExternalInput")
    wq8_d = nc.dram_tensor("wq8", [P, CKD, 2, HPC * D], fp8, kind="ExternalInput")
    wk8_d = nc.dram_tensor("wk8", [P, CKD, 2, HPC * D], fp8, kind="ExternalInput")
    wv8_d = nc.dram_tensor("wv8", [P, CKD, 2, HPC * D], fp8, kind="ExternalInput")
    wpT_d = nc.dram_tensor("wpT", [HPC * D, C], bf16, kind="ExternalInput")
    wfT_d = nc.dram_tensor("wfT", [C, FPC], bf16, kind="ExternalInput")
    wmT_d = nc.dram_tensor("wmT", [FPC, C], bf16, kind="ExternalInput")
    bqk_d = nc.dram_tensor("bqk", [P, 8], f32, kind="ExternalInput")
    bv_d = nc.dram_tensor("bv", [HPC * D], f32, kind="ExternalInput")
    bprojh_d = nc.dram_tensor("bprojh", [C], bf16, kind="ExternalInput")
    bfc_d = nc.dram_tensor("bfc", [P, FK], f32, kind="ExternalInput")
    negmask_d = nc.dram_tensor("negmask", [P, P], f32, kind="ExternalInput")

    out_mlp_d = nc.dram_tensor("out_mlp", [T, C], f32, kind="ExternalOutput")
    out_x2_d = nc.dram_tensor("out_x2", [T, C], f32, kind="ExternalOutput")

    cc_in_d = nc.dram_tensor("cc_in", [T, C], bf16)
    cc_out_d = nc.dram_tensor("cc_out", [T, C], bf16)

    def bcast_row(dram_ap, n):
        return bass.AP(
            tensor=dram_ap.tensor, offset=dram_ap.offset,
            ap=[[0, P], *dram_ap.ap],
        )

    with tile.TileContext(nc, pool_alloc_mode="queue") as tc:
        import contextlib

        with contextlib.ExitStack() as ctx:
            consts = ctx.enter_context(tc.tile_pool(name="consts", bufs=1))
            work = ctx.enter_context(tc.tile_pool(name="work", bufs=2))
            xpool = ctx.enter_context(tc.tile_pool(name="xpool", bufs=1))
            ln_pool = ctx.enter_context(tc.tile_pool(name="ln", bufs=2))
            small = ctx.enter_context(tc.tile_pool(name="small", bufs=1))
            denp = ctx.enter_context(tc.tile_pool(name="denp", bufs=2))
            ppool = ctx.enter_context(tc.tile_pool(name="psum", bufs=1, space="PSUM"))
            scpool = ctx.enter_context(
                tc.tile_pool(name="psum_sc", bufs=5, space="PSUM"))
            pvpool = ctx.enter_context(
                tc.tile_pool(name="psum_pv", bufs=1, space="PSUM"))

            # ---- constants ----
            negmask_sb = consts.tile([P, P], f32)
            nc.scalar.dma_start(negmask_sb[:], negmask_d[:])
            bqk_sb = consts.tile([P, 8], f32)
            nc.scalar.dma_start(bqk_sb[:], bqk_d[:])
            bfc_sb = consts.tile([P, FK], f32)
            nc.scalar.dma_start(bfc_sb[:], bfc_d[:])
            bproj_sb = consts.tile([P, C], bf16)
            nc.scalar.dma_start(bproj_sb[:], bcast_row(bprojh_d[:], C))
            magic_sb = consts.tile([P, 4], i32)
            nc.vector.memset(magic_sb[:], MAGIC)
            dummy_sb = consts.tile([P, 1], f32)
            nc.vector.memset(dummy_sb[:], 0.0)
            # preload the exp activation table while startup DMAs run
            nc.scalar.activation(
                out=dummy_sb[:], in_=dummy_sb[:], func=AF.Exp, scale=1.0)

            def rsqrt_newton(v_ap, n, tag):
                # v_ap: [P, n] fp32 (possibly strided); returns [P, n] ~1/sqrt(v)
                y = ln_pool.tile([P, n], f32, tag=tag + "_y")
                t = ln_pool.tile([P, n], f32, tag=tag + "_t")
                yb = y[:].bitcast(i32)
                nc.vector.tensor_scalar(
                    out=yb, in0=v_ap.bitcast(i32), scalar1=1, scalar2=None,
                    op0=ALU.logical_shift_right,
                )
                nc.vector.tensor_tensor(
                    out=yb, in0=magic_sb[:, :n], in1=yb, op=ALU.subtract)
                for _ in range(2):
                    nc.vector.tensor_tensor(
                        out=t[:], in0=y[:], in1=y[:], op=ALU.mult)
                    nc.vector.tensor_tensor(
                        out=t[:], in0=t[:], in1=v_ap, op=ALU.mult)
                    nc.vector.tensor_scalar(
                        out=t[:], in0=t[:], scalar1=-0.5, scalar2=1.5,
                        op0=ALU.mult, op1=ALU.add,
                    )
                    nc.vector.tensor_tensor(
                        out=y[:], in0=y[:], in1=t[:], op=ALU.mult)
                return y

            # persistent activation tensors (released before MLP)
            attn_cm = tc.tile_pool(name="attn", bufs=1)
            attn_pool = attn_cm.__enter__()
            QT = attn_pool.tile([P, EK, T], bf16)
            KT = attn_pool.tile([P, EK, T], bf16)
            V_aug = attn_pool.tile([P, NT, HPC, 72], fp8)
            OT = attn_pool.tile([P, EK, T], bf16)
            nc.vector.memset(V_aug[:, :, :, D : D + 1], 1.0)

            wp_cm = tc.tile_pool(name="wp", bufs=1)
            wp_pool = wp_cm.__enter__()
            wp_sb = wp_pool.tile([P, EK, C], bf16)
            nc.scalar.dma_start(
                wp_sb[:], wpT_d.ap().rearrange("(k p) o -> p k o", p=P))

            pt_cm = tc.tile_pool(name="ptp", bufs=4)
            pt_pool = pt_cm.__enter__()

            wearly_cm = tc.tile_pool(name="wearly", bufs=1)
            wearly = wearly_cm.__enter__()
            wq_sb = wearly.tile([P, CKD, 2, HPC * D], fp8)
            wk_sb = wearly.tile([P, CKD, 2, HPC * D], fp8)
            wv_sb = wearly.tile([P, CKD, 2, HPC * D], fp8)
            nc.gpsimd.dma_start(wv_sb[:], wv8_d.ap())
            nc.scalar.dma_start(wq_sb[:], wq8_d.ap())
            nc.scalar.dma_start(wk_sb[:], wk8_d.ap())
            xnT_cm = tc.tile_pool(name="p_xnT", bufs=1)
            p_xnT = xnT_cm.__enter__()
            xnTb_cm = tc.tile_pool(name="p_xnTb", bufs=1)
            p_xnTb = xnTb_cm.__enter__()
            # normalized x: bf16 transposed per-run buffer (f = 128ck + p),
            # cast to fp8 xnT8; DoubleRow k-tiles are chunk pairs (2j, 2j+1)
            xnT8 = p_xnT.tile([P, CK, T], fp8)


            xn2T_cm = tc.tile_pool(name="p_xn2T", bufs=1, side="right")
            p_xn2T = xn2T_cm.__enter__()
            xn2T = p_xn2T.tile([P, CK, T], bf16)

            def xnT8_dr(j, t0, n):
                # [P, 2, n] fp8 chunk-pair view for DoubleRow matmuls
                return xnT8[:, 2 * j : 2 * j + 2, t0 : t0 + n]

            def emit_ln1_run(rr):
                # LN1 for 4 tiles of run rr -> xnT8 (fp8, transposed)
                mvb = ln_pool.tile([P, 4, 2], f32, tag="ln1_mv")
                xr = xpool.tile([P, 4, C], f32, tag="xres")
                nc.sync.dma_start(
                    xr[:],
                    x_d[rr * 512 : (rr + 1) * 512, :].rearrange(
                        "(t p) c -> p t c", p=P),
                )
                xs = []
                for i in range(4):
                    x_sb = xr[:, i, :]
                    xg = x_sb.rearrange("p (g f) -> p g f", f=512)
                    stats = ln_pool.tile([P, 2, 6], f32, tag="ln1_st")
                    for g in range(2):
                        nc.vector.bn_stats(out=stats[:, g, :], in_=xg[:, g, :])
                    nc.vector.bn_aggr(out=mvb[:, i, :], in_=stats[:])
                    xs.append(x_sb)
                rstd = rsqrt_newton(mvb[:, :, 1], 4, "ln1")
                xnTb = p_xnTb.tile([P, CK, 512], bf16, tag="xnTb")
                for i in range(4):
                    tt = 4 * rr + i
                    xn_bf = work.tile([P, C], bf16, tag="xn8")
                    nc.vector.tensor_scalar(
                        out=xn_bf[:], in0=xs[i],
                        scalar1=mvb[:, i, 0:1], scalar2=rstd[:, i : i + 1],
                        op0=ALU.subtract, op1=ALU.mult,
                    )
                    nc.sync.dma_start_transpose(
                        xnTb[:, :, i * P : (i + 1) * P], xn_bf[:])
                    nc.vector.tensor_copy(
                        out=xnT8[:, :, (4 * rr + i) * P : (4 * rr + i + 1) * P],
                        in_=xnTb[:, :, i * P : (i + 1) * P],
                    )

            def emit_v_tile(tt):
                ps = ppool.tile([P, 512], f32, tag="mm")
                for ck in range(CKD):
                    nc.tensor.matmul(
                        ps[:],
                        xnT8_dr(ck, tt * P, P),
                        wv_sb[:, ck, :, :],
                        start=(ck == 0), stop=(ck == CKD - 1),
                        perf_mode=DR,
                    )
                nc.vector.tensor_copy(
                    out=V_aug[:, tt, :, 0:D],
                    in_=ps[:].rearrange("p (h e) -> p h e", h=HPC),
                )

            def emit_qk_tile(r, ot):
                # ot in 0..8: 0-3 Q tiles, 4-7 K tiles (natural 128-col chunks)
                w_sb = wq_sb if ot < 4 else wk_sb
                dst = QT if ot < 4 else KT
                ti = ot % 4
                ps = ppool.tile([P, 512], f32, tag="mm")
                for ck in range(CKD):
                    nc.tensor.matmul(
                        ps[:],
                        w_sb[:, ck, :, ti * P : (ti + 1) * P],
                        xnT8_dr(ck, r * 512, 512),
                        start=(ck == 0), stop=(ck == CKD - 1),
                        perf_mode=DR,
                    )
                nc.vector.tensor_scalar(
                    out=dst[:, ti, r * 512 : (r + 1) * 512],
                    in0=ps[:], scalar1=bqk_sb[:, ot : ot + 1], scalar2=None,
                    op0=ALU.add,
                )

            def emit_x2_run(rr):
                # x2 = x + cc (attn partial sum incl b_proj); LN2; transpose
                mvb = ln_pool.tile([P, 4, 2], f32, tag="ln2_mv")
                xr = xpool.tile([P, 4, C], f32, tag="xres")
                dslice = x_d[rr * 512 : (rr + 1) * 512, :].rearrange(
                    "(t p) c -> p t c", p=P)
                nc.sync.dma_start(xr[:], dslice)
                att_sb = xpool.tile([P, 4, C], bf16, tag="attres")
                nc.sync.dma_start(
                    att_sb[:],
                    cc_out_d[rr * 512 : (rr + 1) * 512, :].rearrange(
                        "(t p) c -> p t c", p=P),
                )
                nc.vector.tensor_add(out=xr[:], in0=xr[:], in1=att_sb[:])
                bproj_b4 = bass.AP(
                    tensor=bproj_sb[:].tensor, offset=bproj_sb[:].offset,
                    ap=[bproj_sb[:].ap[0], [0, 4], *bproj_sb[:].ap[1:]],
                )
                nc.vector.tensor_add(out=xr[:], in0=xr[:], in1=bproj_b4)
                nc.sync.dma_start(
                    out_x2_d[rr * 512 : (rr + 1) * 512, :].rearrange(
                        "(t p) c -> p t c", p=P),
                    xr[:],
                )
                x2s = []
                for i in range(4):
                    x_sb = xr[:, i, :]
                    xg = x_sb.rearrange("p (g f) -> p g f", f=512)
                    stats = ln_pool.tile([P, 2, 6], f32, tag="ln2_st")
                    for g in range(2):
                        nc.vector.bn_stats(out=stats[:, g, :], in_=xg[:, g, :])
                    nc.vector.bn_aggr(out=mvb[:, i, :], in_=stats[:])
                    x2s.append(x_sb)
                rstd = rsqrt_newton(mvb[:, :, 1], 4, "ln2")
                for i in range(4):
                    tt = 4 * rr + i
                    xn2_bf = work.tile([P, C], bf16, tag="xn2bf")
                    nc.vector.tensor_scalar(
                        out=xn2_bf[:], in0=x2s[i],
                        scalar1=mvb[:, i, 0:1], scalar2=rstd[:, i : i + 1],
                        op0=ALU.subtract, op1=ALU.mult,
                    )
                    nc.sync.dma_start_transpose(
                        xn2T[:, :, tt * P : (tt + 1) * P], xn2_bf[:])

            # ======== fused pipeline over the 4 token runs ========
            for r in range(NR):
                if r == 0:
                    emit_ln1_run(0)
                    for tt in range(4):
                        emit_v_tile(tt)
                    for ot in range(8):
                        emit_qk_tile(0, ot)
                fillers = []
                if r < NR - 1:
                    emit_ln1_run(r + 1)
                    for tt in range(4 * (r + 1), 4 * (r + 1) + 4):
                        fillers.append(lambda tt=tt: emit_v_tile(tt))
                    for ot in range(8):
                        fillers.append(lambda ot=ot: emit_qk_tile(r + 1, ot))

                # --- attention: heads processed in interleaved pairs so the
                # tensor engine always has independent work while exp runs ---
                ns = 4 * r + 4
                npairs = ns // 2
                pending_mul = []

                def emit_sc(h, st):
                    hp = (h % 2) * D
                    hc = h // 2
                    sc = scpool.tile([P, 512], f32, tag="sc")
                    nc.tensor.matmul(
                        sc[:],
                        KT[hp : hp + D, hc, st * P : (st + 1) * P],
                        QT[hp : hp + D, hc, r * 512 : (r + 1) * 512],
                        start=True, stop=True,
                    )
                    return sc

                def emit_exp(st, sc, PT):
                    j = st - 4 * r
                    off = (st % 2) * 512
                    if j < 0:
                        nc.scalar.activation(
                            out=PT[:, off : off + 512], in_=sc[:],
                            func=AF.Exp, scale=0.125)
                    else:
                        nc.vector.tensor_add(
                            out=sc[:, j * P : (j + 1) * P],
                            in0=sc[:, j * P : (j + 1) * P],
                            in1=negmask_sb[:],
                        )
                        nc.scalar.activation(
                            out=PT[:, off + j * P : off + 512],
                            in_=sc[:, j * P : 512],
                            func=AF.Exp, scale=0.125)
                        if j > 0:
                            nc.gpsimd.memset(PT[:, off : off + j * P], 0.0)

                for h0 in range(0, HPC, 2):
                    heads = (h0, h0 + 1)
                    po_a = pvpool.tile([P, 512], f32, tag="pv0")
                    po_b = pvpool.tile([P, 512], f32, tag="pv1")
                    pos = [po_a, po_b]
                    PTs = [None, None]
                    buf = {}
                    for x in range(2):
                        buf[(x, 0)] = emit_sc(heads[x], 0)
                    for st in range(ns):
                        for x in range(2):
                            if st + 1 < ns:
                                buf[(x, st + 1)] = emit_sc(heads[x], st + 1)
                        if st == 0:
                            while pending_mul:
                                pending_mul.pop(0)()
                        for x in range(2):
                            if st % 2 == 0:
                                PT_new = pt_pool.tile([P, 1024], fp8, tag="PT")
                                PTs[x] = PT_new
                            emit_exp(st, buf.pop((x, st)), PTs[x])
                        if st % 2 == 1:
                            pi = st // 2
                            for x in range(2):
                                nc.tensor.matmul(
                                    pos[x][: D + 1, :],
                                    V_aug[:, 2 * pi : 2 * pi + 2,
                                          heads[x], 0 : D + 1],
                                    PTs[x][:].rearrange(
                                        "p (two n) -> p two n", two=2),
                                    start=(pi == 0), stop=(pi == npairs - 1),
                                    perf_mode=DR,
                                )
                    # denominators for both heads (multiplies deferred into
                    # the next pair so they never block its exp chain)
                    for x in range(2):
                        h = heads[x]
                        hp = (h % 2) * D
                        hc = h // 2
                        po = pos[x]
                        dsum = small.tile([1, 512], f32, tag="dsum")
                        nc.scalar.activation(
                            out=dsum[:], in_=po[D : D + 1, :], func=AF.Identity)
                        rec = small.tile([1, 512], f32, tag="rec")
                        nc.vector.reciprocal_approx_fast(out=rec[:], in_=dsum[:])
                        den = denp.tile([D, 512], f32, tag="den")
                        nc.gpsimd.partition_broadcast(den[:], rec[:])

                        def ot_mul(po=po, den=den, hp=hp, hc=hc):
                            nc.vector.tensor_mul(
                                out=OT[hp : hp + D, hc, r * 512 : (r + 1) * 512],
                                in0=po[0:D, :],
                                in1=den[:],
                            )
                        pending_mul.append(ot_mul)

                    # drain filler tensor work (next run's V/QK)
                    npair_left = (HPC - h0) // 2
                    take = (len(fillers) + npair_left - 1) // npair_left
                    for _ in range(take):
                        fillers.pop(0)()

                while pending_mul:
                    pending_mul.pop(0)()

                # --- c_proj partial (+bproj/2) + AllReduce chunk ---
                for tt in range(4 * r, 4 * r + 4):
                    cc_sb = work.tile([P, C], bf16, tag="ccbuf")
                    for half in range(2):
                        ps = ppool.tile([P, 512], f32, tag="mm")
                        for ek in range(EK):
                            nc.tensor.matmul(
                                ps[:],
                                OT[:, ek, tt * P : (tt + 1) * P],
                                wp_sb[:, ek, half * 512 : (half + 1) * 512],
                                start=(ek == 0), stop=(ek == EK - 1),
                            )
                        nc.vector.tensor_copy(
                            out=cc_sb[:, half * 512 : (half + 1) * 512],
                            in_=ps[:],
                        )
                    nc.sync.dma_start(
                        cc_in_d[tt * P : (tt + 1) * P, :], cc_sb[:])

                nc.gpsimd.collective_compute(
                    "AllReduce",
                    ALU.add,
                    replica_groups=[[0, 1], [2, 3], [4, 5], [6, 7]],
                    ins=[cc_in_d[r * 512 : (r + 1) * 512, :].opt()],
                    outs=[cc_out_d[r * 512 : (r + 1) * 512, :].opt()],
                )
                if r == NR - 1:
                    emit_x2_run(0)
                    emit_x2_run(1)
                if r == NR - 2:
                    # run-3 QKV work is already emitted; free its inputs and
                    # prefetch the first fc weight chunks during run 3
                    xnTb_cm.__exit__(None, None, None)
                    xnT_cm.__exit__(None, None, None)
                    wearly_cm.__exit__(None, None, None)
                    wfe_cm = tc.tile_pool(name="wfearly", bufs=1, side="right")
                    wfe = wfe_cm.__enter__()
                    wfA = wfe.tile([P, 5, FPC], bf16)
                    wfT_r = wfT_d.ap().rearrange("(k p) o -> p k o", p=P)
                    for ck in range(5):
                        eng = nc.scalar if ck % 2 == 0 else nc.gpsimd
                        eng.dma_start(wfA[:, ck, :], wfT_r[:, ck, :])

            # release attention-phase SBUF before the MLP phase
            pt_cm.__exit__(None, None, None)
            wp_cm.__exit__(None, None, None)
            attn_cm.__exit__(None, None, None)

            with tc.tile_pool(name="wlate", bufs=1, side="right") as wlate, \
                 tc.tile_pool(name="p_hT", bufs=1, side="right") as p_hT:
                wfB = wlate.tile([P, CK - 5, FPC], bf16)
                wfT_r = wfT_d.ap().rearrange("(k p) o -> p k o", p=P)
                for ck in range(5, CK):
                    eng = nc.scalar if ck % 2 == 0 else nc.gpsimd
                    eng.dma_start(wfB[:, ck - 5, :], wfT_r[:, ck, :])
                wm_sb = wlate.tile([P, FK, C], bf16)
                wmT_r = wmT_d.ap().rearrange("(k p) o -> p k o", p=P)
                for fk in range(0, FK, 4):
                    eng = nc.scalar if (fk // 4) % 2 == 0 else nc.gpsimd
                    eng.dma_start(
                        wm_sb[:, fk : fk + 4, :], wmT_r[:, fk : fk + 4, :])

                # ======== MLP in 4 token quarters ========
                for tq in range(4):
                    if tq in (0, 1):
                        emit_x2_run(tq + 2)
                    t0 = tq * 512
                    hT = p_hT.tile([P, FK, 512], bf16, tag="hT")
                    for ft in range(FK):
                        ps = ppool.tile([P, 512], f32, tag="mm")
                        for ck in range(CK):
                            wsl = (wfA[:, ck, ft * P : (ft + 1) * P]
                                   if ck < 5 else
                                   wfB[:, ck - 5, ft * P : (ft + 1) * P])
                            nc.tensor.matmul(
                                ps[:],
                                wsl,
                                xn2T[:, ck, t0 : t0 + 512],
                                start=(ck == 0), stop=(ck == CK - 1),
                            )
                        nc.scalar.activation(
                            out=hT[:, ft, :], in_=ps[:],
                            func=AF.Gelu_apprx_tanh,
                            bias=bfc_sb[:, ft : ft + 1], scale=1.0,
                        )
                    for tl in range(4):
                        out_sb = work.tile([P, C], f32, tag="f32buf")
                        for half in range(2):
                            ps = ppool.tile([P, 512], f32, tag="mm")
                            for fk in range(FK):
                                nc.tensor.matmul(
                                    ps[:],
                                    hT[:, fk, tl * P : (tl + 1) * P],
                                    wm_sb[:, fk, half * 512 : (half + 1) * 512],
                                    start=(fk == 0), stop=(fk == FK - 1),
                                )
                            nc.vector.tensor_copy(
                                out=out_sb[:, half * 512 : (half + 1) * 512],
                                in_=ps[:],
                            )
                        nc.gpsimd.dma_start(
                            out_mlp_d[t0 + tl * P : t0 + (tl + 1) * P, :],
                            out_sb[:],
                        )

            wfe_cm.__exit__(None, None, None)
            xn2T_cm.__exit__(None, None, None)

    nc.finalize()
    return nc


def _prep_inputs(x, w_attn, b_attn, w_proj, b_proj, w_fc, b_fc, w_mlp_proj):
    bf = ml_dtypes.bfloat16
    f8 = ml_dtypes.float8_e4m3
    negmask = np.where(
        np.triu(np.ones((P, P), dtype=np.float32)) > 0, 0.0, -1e5
    ).astype(np.float32)

    # lhsT column permutation for Q/K tiles: tile = 2g+ktd, col m ->
    # row 64*(4g + m//32) + 32*ktd + m%32 of the local weight slice
    tiles = np.arange(4)
    m = np.arange(P)
    g = tiles // 2
    ktd = tiles % 2
    rows = (64 * (4 * g[:, None] + m[None, :] // 32)
            + 32 * ktd[:, None] + m[None, :] % 32)  # [4, 128]
    qk_rows = rows.reshape(-1)  # [512]

    def dr_pack(wl, permute):
        # wl [512 out, 1024 feat] -> [128p, 4ck, 2kt, 512 out] fp8
        # feature f = 256*ck + 2*p + kt
        if permute:
            wl = wl[qk_rows, :]
        w4 = wl.reshape(512, CKD, 2, P)          # [out, j, kt, p]
        return np.ascontiguousarray(w4.transpose(3, 1, 2, 0)).astype(f8)

    in_maps = []
    for core in range(8):
        b, s = divmod(core, 2)
        wq = w_attn[s * 512 : (s + 1) * 512, :]
        wk = w_attn[C + s * 512 : C + (s + 1) * 512, :]
        wv = w_attn[2 * C + s * 512 : 2 * C + (s + 1) * 512, :]
        bq = b_attn[s * 512 : (s + 1) * 512]
        bk = b_attn[C + s * 512 : C + (s + 1) * 512]
        bv = b_attn[2 * C + s * 512 : 2 * C + (s + 1) * 512]
        bqk = np.concatenate(
            [bq.reshape(EK, P).T, bk.reshape(EK, P).T], axis=1
        ).astype(np.float32)  # [128, 8] (4 Q tiles, 4 K tiles)
        wp = np.ascontiguousarray(w_proj[:, s * 512 : (s + 1) * 512].T).astype(bf)
        wf = np.ascontiguousarray(w_fc[s * FPC : (s + 1) * FPC, :].T).astype(bf)
        bfc = np.ascontiguousarray(
            b_fc[s * FPC : (s + 1) * FPC].reshape(FK, P).T).astype(np.float32)
        wm = np.ascontiguousarray(
            w_mlp_proj[:, s * FPC : (s + 1) * FPC].T).astype(bf)
        in_maps.append(
            {
                "x": np.ascontiguousarray(x[b]),
                "wq8": dr_pack(wq, False),
                "wk8": dr_pack(wk, False),
                "wv8": dr_pack(wv, False),
                "wpT": wp, "wfT": wf, "wmT": wm,
                "bqk": np.ascontiguousarray(bqk),
                "bv": np.ascontiguousarray(bv).astype(np.float32),
                "bprojh": (b_proj + w_proj @ b_attn[2 * C : 3 * C]).astype(bf),
                "bfc": bfc, "negmask": negmask,
            }
        )
    return in_maps


def run(x, w_attn, b_attn, w_proj, b_proj, w_fc, b_fc, w_mlp_proj, b_mlp_proj,
        trace=False):
    from concourse.bass_utils import run_bass_kernel_spmd

    if "nc" not in _CACHED:
        _CACHED["nc"] = _build_nc()
    nc = _CACHED["nc"]
    in_maps = _prep_inputs(
        x, w_attn, b_attn, w_proj, b_proj, w_fc, b_fc, w_mlp_proj
    )
    res = run_bass_kernel_spmd(
        nc, in_maps, core_ids=list(range(8)), trace=trace,
        trace_cores=list(range(8)) if trace else None,
    )
    out = np.empty((B, T, C), dtype=np.float32)
    for b in range(B):
        a = res.results[2 * b]
        c2 = res.results[2 * b + 1]
        out[b] = a["out_x2"] + a["out_mlp"] + c2["out_mlp"] + b_mlp_proj[None, :]
    return out, res


def kernel(x, w_attn, b_attn, w_proj, b_proj, w_fc, b_fc, w_mlp_proj, b_mlp_proj):
    out, _ = run(
        np.asarray(x, dtype=np.float32),
        np.asarray(w_attn, dtype=np.float32),
        np.asarray(b_attn, dtype=np.float32),
        np.asarray(w_proj, dtype=np.float32),
        np.asarray(b_proj, dtype=np.float32),
        np.asarray(w_fc, dtype=np.float32),
        np.asarray(b_fc, dtype=np.float32),
        np.asarray(w_mlp_proj, dtype=np.float32),
        np.asarray(b_mlp_proj, dtype=np.float32),
    )
    return out


# revision 30
# speedup vs baseline: 1.1748x; 1.1748x over previous
"""Trainium2 Bass kernel for a GPT-style transformer block.

Problem: nn_Block_36807869727037 (dense_transformer)
  B=4, T=2048, C=1024, H=16 heads (d=64), fp32 I/O.
  y = x + attn(LN1(x)); y = y + mlp(LN2(y))  (causal attention, tanh-GELU MLP)

Sharding (8 cores, one uniform SPMD program):
  core = 2*b + s  -> batch b in [0,4), tensor-parallel shard s in [0,2).
  Shard s owns heads [8s, 8s+8) and FFN hidden slice [2048s, 2048s+2048).
  Each core runs the full sequence (T=2048) for its batch:
    LN1 (duplicated in pair) -> QKV for its 8 heads -> causal attention ->
    c_proj partial (+b_proj/2) -> pairwise AllReduce (bf16) -> x2 = x + cc ->
    LN2 (duplicated) -> fc half + GELU -> mlp_proj partial.
  Final combine on host: out[b] = x2 (from core 2b) + mlp_partial(2b)
                                  + mlp_partial(2b+1) + b_mlp_proj.

Precision: QKV projection, scores (QK^T) and PV run in fp8e4 with
DoubleRow perf mode (2x PE throughput); c_proj and the MLP stay bf16.
PSUM accumulation is fp32 everywhere. Scores are computed transposed
S^T[k,q]; softmax uses no max-subtraction (|scores/8| < ~3); the causal
mask is applied additively on PSUM scores BEFORE exp; the denominator
comes from an appended ones-column in V and is applied via DVE
reciprocal + GpSimd partition-broadcast (no tensor-engine involvement).
LayerNorm rstd uses a DVE-only Newton rsqrt so the scalar engine keeps
the exp activation table resident for the whole attention phase.
"""

import sys

sys.path.insert(0, "/opt/trn_rl_repo")

import numpy as np
import ml_dtypes

B, T, C, H = 4, 2048, 1024, 16
D = C // H          # 64 head dim
HPC = H // 2        # 8 heads per core
FPC = 2 * C         # 2048 ffn hidden per core
P = 128
NT = T // P         # 16 token tiles
NR = T // 512       # 4 query runs of 512
CK = C // P         # 8 feature chunks (bf16 path)
CKD = C // 256      # 4 double-row feature chunks (fp8 path)
FK = FPC // P       # 16 ffn chunks per core
EK = (HPC * D) // P  # 4 head-dim chunks per core (512/128)
MAGIC = 0x5F3759DF

_CACHED = {}


def _build_nc():
    import concourse.bass as bass
    import concourse.mybir as mybir
    import concourse.tile as tile
    from concourse import bacc

    f32 = mybir.dt.float32
    bf16 = mybir.dt.bfloat16
    fp8 = mybir.dt.float8e4
    u16 = mybir.dt.uint16
    i32 = mybir.dt.int32
    AF = mybir.ActivationFunctionType
    ALU = mybir.AluOpType
    DR = mybir.MatmulPerfMode.DoubleRow

    nc = bacc.Bacc(trn_type="TRN2", target_bir_lowering=False, num_devices=8)

    # ---- I/O ----
    x_d = nc.dram_tensor("x", [T, C], f32, kind="ExternalInput")
    wq8_d = nc.dram_tensor("wq8", [P, CKD, 2, HPC * D], fp8, kind="ExternalInput")
    wk8_d = nc.dram_tensor("wk8", [P, CKD, 2, HPC * D], fp8, kind="ExternalInput")
    wv8_d = nc.dram_tensor("wv8", [P, CKD, 2, HPC * D], fp8, kind="ExternalInput")
    wpT_d = nc.dram_tensor("wpT", [HPC * D, C], bf16, kind="ExternalInput")
    wfT_d = nc.dram_tensor("wfT", [C, FPC], bf16, kind="ExternalInput")
    wmT_d = nc.dram_tensor("wmT", [FPC, C], bf16, kind="ExternalInput")
    bqk_d = nc.dram_tensor("bqk", [P, 8], f32, kind="ExternalInput")
    bv_d = nc.dram_tensor("bv", [HPC * D], f32, kind="ExternalInput")
    bprojh_d = nc.dram_tensor("bprojh", [C], bf16, kind="ExternalInput")
    bfc_d = nc.dram_tensor("bfc", [P, FK], f32, kind="ExternalInput")
    negmask_d = nc.dram_tensor("negmask", [P, P], f32, kind="ExternalInput")

    out_mlp_d = nc.dram_tensor("out_mlp", [T, C], f32, kind="ExternalOutput")
    out_x2_d = nc.dram_tensor("out_x2", [T, C], f32, kind="ExternalOutput")

    cc_in_d = nc.dram_tensor("cc_in", [T, C], bf16)
    cc_out_d = nc.dram_tensor("cc_out", [T, C], bf16)

    def bcast_row(dram_ap, n):
        return bass.AP(
            tensor=dram_ap.tensor, offset=dram_ap.offset,
            ap=[[0, P], *dram_ap.ap],
        )

    with tile.TileContext(nc, pool_alloc_mode="queue") as tc:
        import contextlib

        with contextlib.ExitStack() as ctx:
            consts = ctx.enter_context(tc.tile_pool(name="consts", bufs=1))
            work = ctx.enter_context(tc.tile_pool(name="work", bufs=2))
            xpool = ctx.enter_context(tc.tile_pool(name="xpool", bufs=1))
            ln_pool = ctx.enter_context(tc.tile_pool(name="ln", bufs=2))
            small = ctx.enter_context(tc.tile_pool(name="small", bufs=1))
            denp = ctx.enter_context(tc.tile_pool(name="denp", bufs=2))
            ppool = ctx.enter_context(tc.tile_pool(name="psum", bufs=2, space="PSUM"))
            scpool = ctx.enter_context(
                tc.tile_pool(name="psum_sc", bufs=4, space="PSUM"))
            pvpool = ctx.enter_context(
                tc.tile_pool(name="psum_pv", bufs=1, space="PSUM"))

            # ---- constants ----
            negmask_sb = consts.tile([P, P], f32)
            nc.scalar.dma_start(negmask_sb[:], negmask_d[:])
            bqk_sb = consts.tile([P, 8], f32)
            nc.scalar.dma_start(bqk_sb[:], bqk_d[:])
            bfc_sb = consts.tile([P, FK], f32)
            nc.scalar.dma_start(bfc_sb[:], bfc_d[:])
            bproj_sb = consts.tile([P, C], bf16)
            nc.scalar.dma_start(bproj_sb[:], bcast_row(bprojh_d[:], C))
            magic_sb = consts.tile([P, 4], i32)
            nc.vector.memset(magic_sb[:], MAGIC)
            dummy_sb = consts.tile([P, 1], f32)
            nc.vector.memset(dummy_sb[:], 0.0)
            # preload the exp activation table while startup DMAs run
            nc.scalar.activation(
                out=dummy_sb[:], in_=dummy_sb[:], func=AF.Exp, scale=1.0)

            def rsqrt_newton(v_ap, n, tag):
                # v_ap: [P, n] fp32 (possibly strided); returns [P, n] ~1/sqrt(v)
                y = ln_pool.tile([P, n], f32, tag=tag + "_y")
                t = ln_pool.tile([P, n], f32, tag=tag + "_t")
                yb = y[:].bitcast(i32)
                nc.vector.tensor_scalar(
                    out=yb, in0=v_ap.bitcast(i32), scalar1=1, scalar2=None,
                    op0=ALU.logical_shift_right,
                )
                nc.vector.tensor_tensor(
                    out=yb, in0=magic_sb[:, :n], in1=yb, op=ALU.subtract)
                for _ in range(2):
                    nc.vector.tensor_tensor(
                        out=t[:], in0=y[:], in1=y[:], op=ALU.mult)
                    nc.vector.tensor_tensor(
                        out=t[:], in0=t[:], in1=v_ap, op=ALU.mult)
                    nc.vector.tensor_scalar(
                        out=t[:], in0=t[:], scalar1=-0.5, scalar2=1.5,
                        op0=ALU.mult, op1=ALU.add,
                    )
                    nc.vector.tensor_tensor(
                        out=y[:], in0=y[:], in1=t[:], op=ALU.mult)
                return y

            # persistent activation tensors (released before MLP)
            attn_cm = tc.tile_pool(name="attn", bufs=1)
            attn_pool = attn_cm.__enter__()
            QT = attn_pool.tile([P, EK, T], bf16)
            KT = attn_pool.tile([P, EK, T], bf16)
            V_aug = attn_pool.tile([P, NT, HPC, 72], fp8)
            OT = attn_pool.tile([P, EK, T], bf16)
            nc.vector.memset(V_aug[:, :, :, D : D + 1], 1.0)

            wp_cm = tc.tile_pool(name="wp", bufs=1)
            wp_pool = wp_cm.__enter__()
            wp_sb = wp_pool.tile([P, EK, C], bf16)
            nc.scalar.dma_start(
                wp_sb[:], wpT_d.ap().rearrange("(k p) o -> p k o", p=P))

            pt_cm = tc.tile_pool(name="ptp", bufs=4)
            pt_pool = pt_cm.__enter__()

            wearly_cm = tc.tile_pool(name="wearly", bufs=1)
            wearly = wearly_cm.__enter__()
            wq_sb = wearly.tile([P, CKD, 2, HPC * D], fp8)
            wk_sb = wearly.tile([P, CKD, 2, HPC * D], fp8)
            wv_sb = wearly.tile([P, CKD, 2, HPC * D], fp8)
            nc.gpsimd.dma_start(wv_sb[:], wv8_d.ap())
            nc.scalar.dma_start(wq_sb[:], wq8_d.ap())
            nc.scalar.dma_start(wk_sb[:], wk8_d.ap())
            xnT_cm = tc.tile_pool(name="p_xnT", bufs=1)
            p_xnT = xnT_cm.__enter__()
            xnTb_cm = tc.tile_pool(name="p_xnTb", bufs=1)
            p_xnTb = xnTb_cm.__enter__()
            # normalized x: bf16 transposed per-run buffer (f = 128ck + p),
            # cast to fp8 xnT8; DoubleRow k-tiles are chunk pairs (2j, 2j+1)
            xnT8 = p_xnT.tile([P, CK, T], fp8)


            xn2T_cm = tc.tile_pool(name="p_xn2T", bufs=1, side="right")
            p_xn2T = xn2T_cm.__enter__()
            xn2T = p_xn2T.tile([P, CK, T], bf16)

            def xnT8_dr(j, t0, n):
                # [P, 2, n] fp8 chunk-pair view for DoubleRow matmuls
                return xnT8[:, 2 * j : 2 * j + 2, t0 : t0 + n]

            def emit_ln1_dma(rr):
                xr = xpool.tile([P, 4, C], f32, tag="xres")
                nc.sync.dma_start(
                    xr[:],
                    x_d[rr * 512 : (rr + 1) * 512, :].rearrange(
                        "(t p) c -> p t c", p=P),
                )
                xnTb = p_xnTb.tile([P, CK, 512], bf16, tag="xnTb")
                return xr, xnTb

            def emit_ln1_chunk(rr, half, xr, xnTb):
                # LN1 for 2 tiles of run rr -> xnT8 (fp8, transposed)
                mvb = ln_pool.tile([P, 2, 2], f32, tag="ln1_mv")
                for i2 in range(2):
                    i = 2 * half + i2
                    xg = xr[:, i, :].rearrange("p (g f) -> p g f", f=512)
                    stats = ln_pool.tile([P, 2, 6], f32, tag="ln1_st")
                    for g in range(2):
                        nc.vector.bn_stats(out=stats[:, g, :], in_=xg[:, g, :])
                    nc.vector.bn_aggr(out=mvb[:, i2, :], in_=stats[:])
                rstd = rsqrt_newton(mvb[:, :, 1], 2, "ln1")
                for i2 in range(2):
                    i = 2 * half + i2
                    tt = 4 * rr + i
                    xn_bf = work.tile([P, C], bf16, tag="xn8")
                    nc.vector.tensor_scalar(
                        out=xn_bf[:], in0=xr[:, i, :],
                        scalar1=mvb[:, i2, 0:1], scalar2=rstd[:, i2 : i2 + 1],
                        op0=ALU.subtract, op1=ALU.mult,
                    )
                    nc.sync.dma_start_transpose(
                        xnTb[:, :, i * P : (i + 1) * P], xn_bf[:])
                    nc.vector.tensor_copy(
                        out=xnT8[:, :, tt * P : (tt + 1) * P],
                        in_=xnTb[:, :, i * P : (i + 1) * P],
                    )

            def emit_ln1_run(rr):
                xr, xnTb = emit_ln1_dma(rr)
                emit_ln1_chunk(rr, 0, xr, xnTb)
                emit_ln1_chunk(rr, 1, xr, xnTb)

            def emit_v_tile(tt):
                ps = ppool.tile([P, 512], f32, tag="mm")
                for ck in range(CKD):
                    nc.tensor.matmul(
                        ps[:],
                        xnT8_dr(ck, tt * P, P),
                        wv_sb[:, ck, :, :],
                        start=(ck == 0), stop=(ck == CKD - 1),
                        perf_mode=DR,
                    )
                nc.vector.tensor_copy(
                    out=V_aug[:, tt, :, 0:D],
                    in_=ps[:].rearrange("p (h e) -> p h e", h=HPC),
                )

            def emit_qk_tile(r, ot):
                # ot in 0..8: 0-3 Q tiles, 4-7 K tiles (natural 128-col chunks)
                w_sb = wq_sb if ot < 4 else wk_sb
                dst = QT if ot < 4 else KT
                ti = ot % 4
                ps = ppool.tile([P, 512], f32, tag="mm")
                for ck in range(CKD):
                    nc.tensor.matmul(
                        ps[:],
                        w_sb[:, ck, :, ti * P : (ti + 1) * P],
                        xnT8_dr(ck, r * 512, 512),
                        start=(ck == 0), stop=(ck == CKD - 1),
                        perf_mode=DR,
                    )
                nc.vector.tensor_scalar(
                    out=dst[:, ti, r * 512 : (r + 1) * 512],
                    in0=ps[:], scalar1=bqk_sb[:, ot : ot + 1], scalar2=None,
                    op0=ALU.add,
                )

            def emit_x2_run(rr):
                # x2 = x + cc (attn partial sum incl b_proj); LN2; transpose
                mvb = ln_pool.tile([P, 4, 2], f32, tag="ln2_mv")
                xr = xpool.tile([P, 4, C], f32, tag="xres")
                dslice = x_d[rr * 512 : (rr + 1) * 512, :].rearrange(
                    "(t p) c -> p t c", p=P)
                nc.sync.dma_start(xr[:], dslice)
                att_sb = xpool.tile([P, 4, C], bf16, tag="attres")
                nc.sync.dma_start(
                    att_sb[:],
                    cc_out_d[rr * 512 : (rr + 1) * 512, :].rearrange(
                        "(t p) c -> p t c", p=P),
                )
                nc.vector.tensor_add(out=xr[:], in0=xr[:], in1=att_sb[:])
                bproj_b4 = bass.AP(
                    tensor=bproj_sb[:].tensor, offset=bproj_sb[:].offset,
                    ap=[bproj_sb[:].ap[0], [0, 4], *bproj_sb[:].ap[1:]],
                )
                nc.vector.tensor_add(out=xr[:], in0=xr[:], in1=bproj_b4)
                nc.sync.dma_start(
                    out_x2_d[rr * 512 : (rr + 1) * 512, :].rearrange(
                        "(t p) c -> p t c", p=P),
                    xr[:],
                )
                x2s = []
                for i in range(4):
                    x_sb = xr[:, i, :]
                    xg = x_sb.rearrange("p (g f) -> p g f", f=512)
                    stats = ln_pool.tile([P, 2, 6], f32, tag="ln2_st")
                    for g in range(2):
                        nc.vector.bn_stats(out=stats[:, g, :], in_=xg[:, g, :])
                    nc.vector.bn_aggr(out=mvb[:, i, :], in_=stats[:])
                    x2s.append(x_sb)
                rstd = rsqrt_newton(mvb[:, :, 1], 4, "ln2")
                for i in range(4):
                    tt = 4 * rr + i
                    xn2_bf = work.tile([P, C], bf16, tag="xn2bf")
                    nc.vector.tensor_scalar(
                        out=xn2_bf[:], in0=x2s[i],
                        scalar1=mvb[:, i, 0:1], scalar2=rstd[:, i : i + 1],
                        op0=ALU.subtract, op1=ALU.mult,
                    )
                    nc.sync.dma_start_transpose(
                        xn2T[:, :, tt * P : (tt + 1) * P], xn2_bf[:])

            # ======== fused pipeline over the 4 token runs ========
            for r in range(NR):
                if r == 0:
                    emit_ln1_run(0)
                    for tt in range(4):
                        emit_v_tile(tt)
                    for ot in range(8):
                        emit_qk_tile(0, ot)
                fillers = []
                if r < NR - 1:
                    xr_n, xnTb_n = emit_ln1_dma(r + 1)
                    t1 = 4 * (r + 1)
                    fillers.append(
                        lambda: emit_ln1_chunk(r + 1, 0, xr_n, xnTb_n))
                    fillers.append(lambda: emit_v_tile(t1))
                    fillers.append(lambda: emit_v_tile(t1 + 1))
                    fillers.append(
                        lambda: emit_ln1_chunk(r + 1, 1, xr_n, xnTb_n))
                    fillers.append(lambda: emit_v_tile(t1 + 2))
                    fillers.append(lambda: emit_v_tile(t1 + 3))
                    for ot in range(8):
                        fillers.append(lambda ot=ot: emit_qk_tile(r + 1, ot))

                # --- attention: heads processed in interleaved pairs so the
                # tensor engine always has independent work while exp runs ---
                ns = 4 * r + 4
                npairs = ns // 2
                pending_mul = []

                def emit_sc(h, st):
                    hp = (h % 2) * D
                    hc = h // 2
                    sc = scpool.tile([P, 512], f32, tag="sc")
                    nc.tensor.matmul(
                        sc[:],
                        KT[hp : hp + D, hc, st * P : (st + 1) * P],
                        QT[hp : hp + D, hc, r * 512 : (r + 1) * 512],
                        start=True, stop=True,
                    )
                    return sc

                def emit_exp(st, sc, PT):
                    j = st - 4 * r
                    off = (st % 2) * 512
                    if j < 0:
                        nc.scalar.activation(
                            out=PT[:, off : off + 512], in_=sc[:],
                            func=AF.Exp, scale=0.125)
                    else:
                        nc.vector.tensor_add(
                            out=sc[:, j * P : (j + 1) * P],
                            in0=sc[:, j * P : (j + 1) * P],
                            in1=negmask_sb[:],
                        )
                        nc.scalar.activation(
                            out=PT[:, off + j * P : off + 512],
                            in_=sc[:, j * P : 512],
                            func=AF.Exp, scale=0.125)
                        if j > 0:
                            nc.gpsimd.memset(PT[:, off : off + j * P], 0.0)

                for h0 in range(0, HPC, 2):
                    heads = (h0, h0 + 1)
                    po_a = pvpool.tile([P, 512], f32, tag="pv0")
                    po_b = pvpool.tile([P, 512], f32, tag="pv1")
                    pos = [po_a, po_b]
                    PTs = [None, None]
                    buf = {}
                    for x in range(2):
                        buf[(x, 0)] = emit_sc(heads[x], 0)
                    for st in range(ns):
                        for x in range(2):
                            if st + 1 < ns:
                                buf[(x, st + 1)] = emit_sc(heads[x], st + 1)
                        if st == 0:
                            while pending_mul:
                                pending_mul.pop(0)()
                        for x in range(2):
                            if st % 2 == 0:
                                PT_new = pt_pool.tile([P, 1024], fp8, tag="PT")
                                PTs[x] = PT_new
                            emit_exp(st, buf.pop((x, st)), PTs[x])
                        if st % 2 == 1:
                            pi = st // 2
                            for x in range(2):
                                nc.tensor.matmul(
                                    pos[x][: D + 1, :],
                                    V_aug[:, 2 * pi : 2 * pi + 2,
                                          heads[x], 0 : D + 1],
                                    PTs[x][:].rearrange(
                                        "p (two n) -> p two n", two=2),
                                    start=(pi == 0), stop=(pi == npairs - 1),
                                    perf_mode=DR,
                                )
                    # denominators for both heads (multiplies deferred into
                    # the next pair so they never block its exp chain)
                    for x in range(2):
                        h = heads[x]
                        hp = (h % 2) * D
                        hc = h // 2
                        po = pos[x]
                        dsum = small.tile([1, 512], f32, tag="dsum")
                        nc.scalar.activation(
                            out=dsum[:], in_=po[D : D + 1, :], func=AF.Identity)
                        rec = small.tile([1, 512], f32, tag="rec")
                        nc.vector.reciprocal_approx_fast(out=rec[:], in_=dsum[:])
                        den = denp.tile([D, 512], f32, tag="den")
                        nc.gpsimd.partition_broadcast(den[:], rec[:])

                        def ot_mul(po=po, den=den, hp=hp, hc=hc):
                            nc.vector.tensor_mul(
                                out=OT[hp : hp + D, hc, r * 512 : (r + 1) * 512],
                                in0=po[0:D, :],
                                in1=den[:],
                            )
                        pending_mul.append(ot_mul)

                    # drain filler tensor work (next run's V/QK)
                    npair_left = (HPC - h0) // 2
                    take = (len(fillers) + npair_left - 1) // npair_left
                    for _ in range(take):
                        fillers.pop(0)()

                while pending_mul:
                    pending_mul.pop(0)()

                # --- c_proj partial (+bproj/2) + AllReduce chunk ---
                for tt in range(4 * r, 4 * r + 4):
                    cc_sb = work.tile([P, C], bf16, tag="ccbuf")
                    for half in range(2):
                        ps = ppool.tile([P, 512], f32, tag="mm")
                        for ek in range(EK):
                            nc.tensor.matmul(
                                ps[:],
                                OT[:, ek, tt * P : (tt + 1) * P],
                                wp_sb[:, ek, half * 512 : (half + 1) * 512],
                                start=(ek == 0), stop=(ek == EK - 1),
                            )
                        nc.vector.tensor_copy(
                            out=cc_sb[:, half * 512 : (half + 1) * 512],
                            in_=ps[:],
                        )
                    nc.sync.dma_start(
                        cc_in_d[tt * P : (tt + 1) * P, :], cc_sb[:])

                nc.gpsimd.collective_compute(
                    "AllReduce",
                    ALU.add,
                    replica_groups=[[0, 1], [2, 3], [4, 5], [6, 7]],
                    ins=[cc_in_d[r * 512 : (r + 1) * 512, :].opt()],
                    outs=[cc_out_d[r * 512 : (r + 1) * 512, :].opt()],
                )
                if r == NR - 1:
                    emit_x2_run(0)
                    emit_x2_run(1)
                if r == NR - 2:
                    # run-3 QKV work is already emitted; free its inputs and
                    # prefetch the first fc weight chunks during run 3
                    xnTb_cm.__exit__(None, None, None)
                    xnT_cm.__exit__(None, None, None)
                    wearly_cm.__exit__(None, None, None)
                    wfe_cm = tc.tile_pool(name="wfearly", bufs=1, side="right")
                    wfe = wfe_cm.__enter__()
                    wfA = wfe.tile([P, 5, FPC], bf16)
                    wfT_r = wfT_d.ap().rearrange("(k p) o -> p k o", p=P)
                    for ck in range(5):
                        eng = nc.scalar if ck % 2 == 0 else nc.gpsimd
                        eng.dma_start(wfA[:, ck, :], wfT_r[:, ck, :])

            # release attention-phase SBUF before the MLP phase
            pt_cm.__exit__(None, None, None)
            wp_cm.__exit__(None, None, None)
            attn_cm.__exit__(None, None, None)

            with tc.tile_pool(name="wlate", bufs=1, side="right") as wlate, \
                 tc.tile_pool(name="p_hT", bufs=1, side="right") as p_hT:
                wfB = wlate.tile([P, CK - 5, FPC], bf16)
                wfT_r = wfT_d.ap().rearrange("(k p) o -> p k o", p=P)
                for ck in range(5, CK):
                    eng = nc.scalar if ck % 2 == 0 else nc.gpsimd
                    eng.dma_start(wfB[:, ck - 5, :], wfT_r[:, ck, :])
                wm_sb = wlate.tile([P, FK, C], bf16)
                wmT_r = wmT_d.ap().rearrange("(k p) o -> p k o", p=P)
                for fk in range(0, FK, 4):
                    eng = nc.scalar if (fk // 4) % 2 == 0 else nc.gpsimd
                    eng.dma_start(
                        wm_sb[:, fk : fk + 4, :], wmT_r[:, fk : fk + 4, :])

                # ======== MLP in 4 token quarters ========
                for tq in range(4):
                    if tq in (0, 1):
                        emit_x2_run(tq + 2)
                    t0 = tq * 512
                    hT = p_hT.tile([P, FK, 512], bf16, tag="hT")
                    for ft in range(FK):
                        ps = ppool.tile([P, 512], f32, tag="mm")
                        for ck in range(CK):
                            wsl = (wfA[:, ck, ft * P : (ft + 1) * P]
                                   if ck < 5 else
                                   wfB[:, ck - 5, ft * P : (ft + 1) * P])
                            nc.tensor.matmul(
                                ps[:],
                                wsl,
                                xn2T[:, ck, t0 : t0 + 512],
                                start=(ck == 0), stop=(ck == CK - 1),
                            )
                        nc.scalar.activation(
                            out=hT[:, ft, :], in_=ps[:],
                            func=AF.Gelu_apprx_tanh,
                            bias=bfc_sb[:, ft : ft + 1], scale=1.0,
                        )
                    for tl in range(4):
                        out_sb = work.tile([P, C], f32, tag="f32buf")
                        for half in range(2):
                            ps = ppool.tile([P, 512], f32, tag="mm")
                            for fk in range(FK):
                                nc.tensor.matmul(
                                    ps[:],
                                    hT[:, fk, tl * P : (tl + 1) * P],
                                    wm_sb[:, fk, half * 512 : (half + 1) * 512],
                                    start=(fk == 0), stop=(fk == FK - 1),
                                )
                            nc.vector.tensor_copy(
                                out=out_sb[:, half * 512 : (half + 1) * 512],
                                in_=ps[:],
                            )
                        nc.gpsimd.dma_start(
                            out_mlp_d[t0 + tl * P : t0 + (tl + 1) * P, :],
                            out_sb[:],
                        )

            wfe_cm.__exit__(None, None, None)
            xn2T_cm.__exit__(None, None, None)

    nc.finalize()
    return nc


def _prep_inputs(x, w_attn, b_attn, w_proj, b_proj, w_fc, b_fc, w_mlp_proj):
    bf = ml_dtypes.bfloat16
    f8 = ml_dtypes.float8_e4m3
    negmask = np.where(
        np.triu(np.ones((P, P), dtype=np.float32)) > 0, 0.0, -1e5
    ).astype(np.float32)

    # lhsT column permutation for Q/K tiles: tile = 2g+ktd, col m ->
    # row 64*(4g + m//32) + 32*ktd + m%32 of the local weight slice
    tiles = np.arange(4)
    m = np.arange(P)
    g = tiles // 2
    ktd = tiles % 2
    rows = (64 * (4 * g[:, None] + m[None, :] // 32)
            + 32 * ktd[:, None] + m[None, :] % 32)  # [4, 128]
    qk_rows = rows.reshape(-1)  # [512]

    def dr_pack(wl, permute):
        # wl [512 out, 1024 feat] -> [128p, 4ck, 2kt, 512 out] fp8
        # feature f = 256*ck + 2*p + kt
        if permute:
            wl = wl[qk_rows, :]
        w4 = wl.reshape(512, CKD, 2, P)          # [out, j, kt, p]
        return np.ascontiguousarray(w4.transpose(3, 1, 2, 0)).astype(f8)

    in_maps = []
    for core in range(8):
        b, s = divmod(core, 2)
        wq = w_attn[s * 512 : (s + 1) * 512, :]
        wk = w_attn[C + s * 512 : C + (s + 1) * 512, :]
        wv = w_attn[2 * C + s * 512 : 2 * C + (s + 1) * 512, :]
        bq = b_attn[s * 512 : (s + 1) * 512]
        bk = b_attn[C + s * 512 : C + (s + 1) * 512]
        bv = b_attn[2 * C + s * 512 : 2 * C + (s + 1) * 512]
        bqk = np.concatenate(
            [bq.reshape(EK, P).T, bk.reshape(EK, P).T], axis=1
        ).astype(np.float32)  # [128, 8] (4 Q tiles, 4 K tiles)
        wp = np.ascontiguousarray(w_proj[:, s * 512 : (s + 1) * 512].T).astype(bf)
        wf = np.ascontiguousarray(w_fc[s * FPC : (s + 1) * FPC, :].T).astype(bf)
        bfc = np.ascontiguousarray(
            b_fc[s * FPC : (s + 1) * FPC].reshape(FK, P).T).astype(np.float32)
        wm = np.ascontiguousarray(
            w_mlp_proj[:, s * FPC : (s + 1) * FPC].T).astype(bf)
        in_maps.append(
            {
                "x": np.ascontiguousarray(x[b]),
                "wq8": dr_pack(wq, False),
                "wk8": dr_pack(wk, False),
                "wv8": dr_pack(wv, False),
                "wpT": wp, "wfT": wf, "wmT": wm,
                "bqk": np.ascontiguousarray(bqk),
                "bv": np.ascontiguousarray(bv).astype(np.float32),
                "bprojh": (b_proj + w_proj @ b_attn[2 * C : 3 * C]).astype(bf),
                "bfc": bfc, "negmask": negmask,
            }
        )
    return in_maps


def run(x, w_attn, b_attn, w_proj, b_proj, w_fc, b_fc, w_mlp_proj, b_mlp_proj,
        trace=False):
    from concourse.bass_utils import run_bass_kernel_spmd

    if "nc" not in _CACHED:
        _CACHED["nc"] = _build_nc()
    nc = _CACHED["nc"]
    in_maps = _prep_inputs(
        x, w_attn, b_attn, w_proj, b_proj, w_fc, b_fc, w_mlp_proj
    )
    res = run_bass_kernel_spmd(
        nc, in_maps, core_ids=list(range(8)), trace=trace,
        trace_cores=list(range(8)) if trace else None,
    )
    out = np.empty((B, T, C), dtype=np.float32)
    for b in range(B):
        a = res.results[2 * b]
        c2 = res.results[2 * b + 1]
        out[b] = a["out_x2"] + a["out_mlp"] + c2["out_mlp"] + b_mlp_proj[None, :]
    return out, res


def kernel(x, w_attn, b_attn, w_proj, b_proj, w_fc, b_fc, w_mlp_proj, b_mlp_proj):
    out, _ = run(
        np.asarray(x, dtype=np.float32),
        np.asarray(w_attn, dtype=np.float32),
        np.asarray(b_attn, dtype=np.float32),
        np.asarray(w_proj, dtype=np.float32),
        np.asarray(b_proj, dtype=np.float32),
        np.asarray(w_fc, dtype=np.float32),
        np.asarray(b_fc, dtype=np.float32),
        np.asarray(w_mlp_proj, dtype=np.float32),
        np.asarray(b_mlp_proj, dtype=np.float32),
    )
    return out


# revision 31
# speedup vs baseline: 1.3245x; 1.1274x over previous
"""Trainium2 Bass kernel for a GPT-style transformer block.

Problem: nn_Block_36807869727037 (dense_transformer)
  B=4, T=2048, C=1024, H=16 heads (d=64), fp32 I/O.
  y = x + attn(LN1(x)); y = y + mlp(LN2(y))  (causal attention, tanh-GELU MLP)

Sharding (8 cores, one uniform SPMD program):
  core = 2*b + s  -> batch b in [0,4), tensor-parallel shard s in [0,2).
  Shard s owns heads [8s, 8s+8) and FFN hidden slice [2048s, 2048s+2048).
  Each core runs the full sequence (T=2048) for its batch:
    LN1 (duplicated in pair) -> QKV for its 8 heads -> causal attention ->
    c_proj partial (+b_proj/2) -> pairwise AllReduce (bf16) -> x2 = x + cc ->
    LN2 (duplicated) -> fc half + GELU -> mlp_proj partial.
  Final combine on host: out[b] = x2 (from core 2b) + mlp_partial(2b)
                                  + mlp_partial(2b+1) + b_mlp_proj.

Precision: QKV projection, scores (QK^T) and PV run in fp8e4 with
DoubleRow perf mode (2x PE throughput); c_proj and the MLP stay bf16.
PSUM accumulation is fp32 everywhere. Scores are computed transposed
S^T[k,q]; softmax uses no max-subtraction (|scores/8| < ~3); the causal
mask is applied additively on PSUM scores BEFORE exp; the denominator
comes from an appended ones-column in V and is applied via DVE
reciprocal + GpSimd partition-broadcast (no tensor-engine involvement).
LayerNorm rstd uses a DVE-only Newton rsqrt so the scalar engine keeps
the exp activation table resident for the whole attention phase.
"""

import sys

sys.path.insert(0, "/opt/trn_rl_repo")

import numpy as np
import ml_dtypes

B, T, C, H = 4, 2048, 1024, 16
D = C // H          # 64 head dim
HPC = H // 2        # 8 heads per core
FPC = 2 * C         # 2048 ffn hidden per core
P = 128
NT = T // P         # 16 token tiles
NR = T // 512       # 4 query runs of 512
CK = C // P         # 8 feature chunks (bf16 path)
CKD = C // 256      # 4 double-row feature chunks (fp8 path)
FK = FPC // P       # 16 ffn chunks per core
EK = (HPC * D) // P  # 4 head-dim chunks per core (512/128)
MAGIC = 0x5F3759DF

_CACHED = {}


def _build_nc():
    import concourse.bass as bass
    import concourse.mybir as mybir
    import concourse.tile as tile
    from concourse import bacc

    f32 = mybir.dt.float32
    bf16 = mybir.dt.bfloat16
    fp8 = mybir.dt.float8e4
    u16 = mybir.dt.uint16
    i32 = mybir.dt.int32
    AF = mybir.ActivationFunctionType
    ALU = mybir.AluOpType
    DR = mybir.MatmulPerfMode.DoubleRow

    nc = bacc.Bacc(trn_type="TRN2", target_bir_lowering=False, num_devices=8)

    # ---- I/O ----
    x_d = nc.dram_tensor("x", [T, C], f32, kind="ExternalInput")
    wq8_d = nc.dram_tensor("wq8", [P, CKD, 2, HPC * D], fp8, kind="ExternalInput")
    wk8_d = nc.dram_tensor("wk8", [P, CKD, 2, HPC * D], fp8, kind="ExternalInput")
    wv8_d = nc.dram_tensor("wv8", [P, CKD, 2, HPC * D], fp8, kind="ExternalInput")
    wpT_d = nc.dram_tensor("wpT", [HPC * D, C], bf16, kind="ExternalInput")
    wfT_d = nc.dram_tensor("wfT", [C, FPC], bf16, kind="ExternalInput")
    wmT_d = nc.dram_tensor("wmT", [FPC, C], bf16, kind="ExternalInput")
    bqk_d = nc.dram_tensor("bqk", [P, 8], f32, kind="ExternalInput")
    bv_d = nc.dram_tensor("bv", [HPC * D], f32, kind="ExternalInput")
    bprojh_d = nc.dram_tensor("bprojh", [C], bf16, kind="ExternalInput")
    bfc_d = nc.dram_tensor("bfc", [P, FK], f32, kind="ExternalInput")
    negmask_d = nc.dram_tensor("negmask", [P, P], f32, kind="ExternalInput")

    out_mlp_d = nc.dram_tensor("out_mlp", [T, C], f32, kind="ExternalOutput")
    out_x2_d = nc.dram_tensor("out_x2", [T, C], f32, kind="ExternalOutput")

    cc_in_d = nc.dram_tensor("cc_in", [T, C], bf16)
    cc_out_d = nc.dram_tensor("cc_out", [T, C], bf16)

    def bcast_row(dram_ap, n):
        return bass.AP(
            tensor=dram_ap.tensor, offset=dram_ap.offset,
            ap=[[0, P], *dram_ap.ap],
        )

    with tile.TileContext(nc, pool_alloc_mode="queue") as tc:
        import contextlib

        with contextlib.ExitStack() as ctx:
            consts = ctx.enter_context(tc.tile_pool(name="consts", bufs=1))
            work = ctx.enter_context(tc.tile_pool(name="work", bufs=2))
            xpool = ctx.enter_context(tc.tile_pool(name="xpool", bufs=1))
            ln_pool = ctx.enter_context(tc.tile_pool(name="ln", bufs=2))
            small = ctx.enter_context(tc.tile_pool(name="small", bufs=1))
            denp = ctx.enter_context(tc.tile_pool(name="denp", bufs=2))
            ppool = ctx.enter_context(tc.tile_pool(name="psum", bufs=2, space="PSUM"))
            scpool = ctx.enter_context(
                tc.tile_pool(name="psum_sc", bufs=4, space="PSUM"))
            pvpool = ctx.enter_context(
                tc.tile_pool(name="psum_pv", bufs=1, space="PSUM"))

            # ---- constants ----
            negmask_sb = consts.tile([P, P], f32)
            nc.scalar.dma_start(negmask_sb[:], negmask_d[:])
            bqk_sb = consts.tile([P, 8], f32)
            nc.scalar.dma_start(bqk_sb[:], bqk_d[:])
            bfc_sb = consts.tile([P, FK], f32)
            nc.scalar.dma_start(bfc_sb[:], bfc_d[:])
            bproj_sb = consts.tile([P, C], bf16)
            nc.scalar.dma_start(bproj_sb[:], bcast_row(bprojh_d[:], C))
            magic_sb = consts.tile([P, 4], i32)
            nc.vector.memset(magic_sb[:], MAGIC)
            dummy_sb = consts.tile([P, 1], f32)
            nc.vector.memset(dummy_sb[:], 0.0)
            # preload the exp activation table while startup DMAs run
            nc.scalar.activation(
                out=dummy_sb[:], in_=dummy_sb[:], func=AF.Exp, scale=1.0)

            def rsqrt_newton(v_ap, n, tag):
                # v_ap: [P, n] fp32 (possibly strided); returns [P, n] ~1/sqrt(v)
                y = ln_pool.tile([P, n], f32, tag=tag + "_y")
                t = ln_pool.tile([P, n], f32, tag=tag + "_t")
                yb = y[:].bitcast(i32)
                nc.vector.tensor_scalar(
                    out=yb, in0=v_ap.bitcast(i32), scalar1=1, scalar2=None,
                    op0=ALU.logical_shift_right,
                )
                nc.vector.tensor_tensor(
                    out=yb, in0=magic_sb[:, :n], in1=yb, op=ALU.subtract)
                for _ in range(2):
                    nc.vector.tensor_tensor(
                        out=t[:], in0=y[:], in1=y[:], op=ALU.mult)
                    nc.vector.tensor_tensor(
                        out=t[:], in0=t[:], in1=v_ap, op=ALU.mult)
                    nc.vector.tensor_scalar(
                        out=t[:], in0=t[:], scalar1=-0.5, scalar2=1.5,
                        op0=ALU.mult, op1=ALU.add,
                    )
                    nc.vector.tensor_tensor(
                        out=y[:], in0=y[:], in1=t[:], op=ALU.mult)
                return y

            # persistent activation tensors (released before MLP)
            attn_cm = tc.tile_pool(name="attn", bufs=1)
            attn_pool = attn_cm.__enter__()
            QT = attn_pool.tile([P, EK, T], bf16)
            KT = attn_pool.tile([P, EK, T], bf16)
            V_aug = attn_pool.tile([P, NT, HPC, 72], fp8)
            OT = attn_pool.tile([P, EK, T], bf16)
            nc.vector.memset(V_aug[:, :, :, D : D + 1], 1.0)

            wp_cm = tc.tile_pool(name="wp", bufs=1)
            wp_pool = wp_cm.__enter__()
            wp_sb = wp_pool.tile([P, EK, C], bf16)
            nc.scalar.dma_start(
                wp_sb[:], wpT_d.ap().rearrange("(k p) o -> p k o", p=P))

            pt_cm = tc.tile_pool(name="ptp", bufs=4)
            pt_pool = pt_cm.__enter__()

            wearly_cm = tc.tile_pool(name="wearly", bufs=1)
            wearly = wearly_cm.__enter__()
            wq_sb = wearly.tile([P, CKD, 2, HPC * D], fp8)
            wk_sb = wearly.tile([P, CKD, 2, HPC * D], fp8)
            wv_sb = wearly.tile([P, CKD, 2, HPC * D], fp8)
            nc.gpsimd.dma_start(wv_sb[:], wv8_d.ap())
            nc.scalar.dma_start(wq_sb[:], wq8_d.ap())
            nc.scalar.dma_start(wk_sb[:], wk8_d.ap())
            xnT_cm = tc.tile_pool(name="p_xnT", bufs=1)
            p_xnT = xnT_cm.__enter__()
            xnTb_cm = tc.tile_pool(name="p_xnTb", bufs=1)
            p_xnTb = xnTb_cm.__enter__()
            # normalized x: bf16 transposed per-run buffer (f = 128ck + p),
            # cast to fp8 xnT8; DoubleRow k-tiles are chunk pairs (2j, 2j+1)
            xnT8 = p_xnT.tile([P, CK, T], fp8)


            xn2T_cm = tc.tile_pool(name="p_xn2T", bufs=1, side="right")
            p_xn2T = xn2T_cm.__enter__()
            xn2T = p_xn2T.tile([P, CK, T], bf16)

            def xnT8_dr(j, t0, n):
                # [P, 2, n] fp8 chunk-pair view for DoubleRow matmuls
                return xnT8[:, 2 * j : 2 * j + 2, t0 : t0 + n]

            def emit_ln1_dma(rr):
                xr = xpool.tile([P, 4, C], f32, tag="xres")
                nc.sync.dma_start(
                    xr[:],
                    x_d[rr * 512 : (rr + 1) * 512, :].rearrange(
                        "(t p) c -> p t c", p=P),
                )
                xnTb = p_xnTb.tile([P, CK, 512], bf16, tag="xnTb")
                return xr, xnTb

            def emit_ln1_chunk(rr, half, xr, xnTb):
                # LN1 for 2 tiles of run rr -> xnT8 (fp8, transposed)
                mvb = ln_pool.tile([P, 2, 2], f32, tag="ln1_mv")
                for i2 in range(2):
                    i = 2 * half + i2
                    xg = xr[:, i, :].rearrange("p (g f) -> p g f", f=512)
                    stats = ln_pool.tile([P, 2, 6], f32, tag="ln1_st")
                    for g in range(2):
                        nc.vector.bn_stats(out=stats[:, g, :], in_=xg[:, g, :])
                    nc.vector.bn_aggr(out=mvb[:, i2, :], in_=stats[:])
                rstd = rsqrt_newton(mvb[:, :, 1], 2, "ln1")
                for i2 in range(2):
                    i = 2 * half + i2
                    tt = 4 * rr + i
                    xn_bf = work.tile([P, C], bf16, tag="xn8")
                    nc.vector.tensor_scalar(
                        out=xn_bf[:], in0=xr[:, i, :],
                        scalar1=mvb[:, i2, 0:1], scalar2=rstd[:, i2 : i2 + 1],
                        op0=ALU.subtract, op1=ALU.mult,
                    )
                    nc.sync.dma_start_transpose(
                        xnTb[:, :, i * P : (i + 1) * P], xn_bf[:])
                    nc.vector.tensor_copy(
                        out=xnT8[:, :, tt * P : (tt + 1) * P],
                        in_=xnTb[:, :, i * P : (i + 1) * P],
                    )

            def emit_ln1_run(rr):
                xr, xnTb = emit_ln1_dma(rr)
                emit_ln1_chunk(rr, 0, xr, xnTb)
                emit_ln1_chunk(rr, 1, xr, xnTb)

            def emit_v_tile(tt):
                ps = ppool.tile([P, 512], f32, tag="mm")
                for ck in range(CKD):
                    nc.tensor.matmul(
                        ps[:],
                        xnT8_dr(ck, tt * P, P),
                        wv_sb[:, ck, :, :],
                        start=(ck == 0), stop=(ck == CKD - 1),
                        perf_mode=DR,
                    )
                nc.vector.tensor_copy(
                    out=V_aug[:, tt, :, 0:D],
                    in_=ps[:].rearrange("p (h e) -> p h e", h=HPC),
                )

            def emit_qk_tile(r, ot):
                # ot in 0..8: 0-3 Q tiles, 4-7 K tiles (natural 128-col chunks)
                w_sb = wq_sb if ot < 4 else wk_sb
                dst = QT if ot < 4 else KT
                ti = ot % 4
                ps = ppool.tile([P, 512], f32, tag="mm")
                for ck in range(CKD):
                    nc.tensor.matmul(
                        ps[:],
                        w_sb[:, ck, :, ti * P : (ti + 1) * P],
                        xnT8_dr(ck, r * 512, 512),
                        start=(ck == 0), stop=(ck == CKD - 1),
                        perf_mode=DR,
                    )
                nc.vector.tensor_scalar(
                    out=dst[:, ti, r * 512 : (r + 1) * 512],
                    in0=ps[:], scalar1=bqk_sb[:, ot : ot + 1], scalar2=None,
                    op0=ALU.add,
                )

            def emit_x2_run(rr):
                # x2 = x + cc (attn partial sum incl b_proj); LN2; transpose
                mvb = ln_pool.tile([P, 4, 2], f32, tag="ln2_mv")
                xr = xpool.tile([P, 4, C], f32, tag="xres")
                dslice = x_d[rr * 512 : (rr + 1) * 512, :].rearrange(
                    "(t p) c -> p t c", p=P)
                nc.sync.dma_start(xr[:], dslice)
                att_sb = xpool.tile([P, 4, C], bf16, tag="attres")
                nc.sync.dma_start(
                    att_sb[:],
                    cc_out_d[rr * 512 : (rr + 1) * 512, :].rearrange(
                        "(t p) c -> p t c", p=P),
                )
                nc.vector.tensor_add(out=xr[:], in0=xr[:], in1=att_sb[:])
                bproj_b4 = bass.AP(
                    tensor=bproj_sb[:].tensor, offset=bproj_sb[:].offset,
                    ap=[bproj_sb[:].ap[0], [0, 4], *bproj_sb[:].ap[1:]],
                )
                nc.vector.tensor_add(out=xr[:], in0=xr[:], in1=bproj_b4)
                nc.sync.dma_start(
                    out_x2_d[rr * 512 : (rr + 1) * 512, :].rearrange(
                        "(t p) c -> p t c", p=P),
                    xr[:],
                )
                x2s = []
                for i in range(4):
                    x_sb = xr[:, i, :]
                    xg = x_sb.rearrange("p (g f) -> p g f", f=512)
                    stats = ln_pool.tile([P, 2, 6], f32, tag="ln2_st")
                    for g in range(2):
                        nc.vector.bn_stats(out=stats[:, g, :], in_=xg[:, g, :])
                    nc.vector.bn_aggr(out=mvb[:, i, :], in_=stats[:])
                    x2s.append(x_sb)
                rstd = rsqrt_newton(mvb[:, :, 1], 4, "ln2")
                for i in range(4):
                    tt = 4 * rr + i
                    xn2_bf = work.tile([P, C], bf16, tag="xn2bf")
                    nc.vector.tensor_scalar(
                        out=xn2_bf[:], in0=x2s[i],
                        scalar1=mvb[:, i, 0:1], scalar2=rstd[:, i : i + 1],
                        op0=ALU.subtract, op1=ALU.mult,
                    )
                    nc.sync.dma_start_transpose(
                        xn2T[:, :, tt * P : (tt + 1) * P], xn2_bf[:])

            # ======== fused pipeline over the 4 token runs ========
            for r in range(NR):
                if r == 0:
                    xr0, xnTb0 = emit_ln1_dma(0)
                    emit_ln1_chunk(0, 0, xr0, xnTb0)
                    emit_v_tile(0)
                    emit_v_tile(1)
                    emit_ln1_chunk(0, 1, xr0, xnTb0)
                    emit_v_tile(2)
                    emit_v_tile(3)
                    for ot in range(8):
                        emit_qk_tile(0, ot)
                fillers = []
                if r < NR - 1:
                    emit_ln1_run(r + 1)
                    for tt in range(4 * (r + 1), 4 * (r + 1) + 4):
                        fillers.append(lambda tt=tt: emit_v_tile(tt))
                    for ot in range(8):
                        fillers.append(lambda ot=ot: emit_qk_tile(r + 1, ot))

                # --- attention: heads processed in interleaved pairs so the
                # tensor engine always has independent work while exp runs ---
                ns = 4 * r + 4
                npairs = ns // 2
                pending_mul = []

                def emit_sc(h, st):
                    hp = (h % 2) * D
                    hc = h // 2
                    sc = scpool.tile([P, 512], f32, tag="sc")
                    nc.tensor.matmul(
                        sc[:],
                        KT[hp : hp + D, hc, st * P : (st + 1) * P],
                        QT[hp : hp + D, hc, r * 512 : (r + 1) * 512],
                        start=True, stop=True,
                    )
                    return sc

                def emit_exp(st, sc, PT):
                    j = st - 4 * r
                    off = (st % 2) * 512
                    if j < 0:
                        nc.scalar.activation(
                            out=PT[:, off : off + 512], in_=sc[:],
                            func=AF.Exp, scale=0.125)
                    else:
                        nc.vector.tensor_add(
                            out=sc[:, j * P : (j + 1) * P],
                            in0=sc[:, j * P : (j + 1) * P],
                            in1=negmask_sb[:],
                        )
                        nc.scalar.activation(
                            out=PT[:, off + j * P : off + 512],
                            in_=sc[:, j * P : 512],
                            func=AF.Exp, scale=0.125)
                        if j > 0:
                            nc.gpsimd.memset(PT[:, off : off + j * P], 0.0)

                for h0 in range(0, HPC, 2):
                    heads = (h0, h0 + 1)
                    po_a = pvpool.tile([P, 512], f32, tag="pv0")
                    po_b = pvpool.tile([P, 512], f32, tag="pv1")
                    pos = [po_a, po_b]
                    PTs = [None, None]
                    buf = {}
                    for x in range(2):
                        buf[(x, 0)] = emit_sc(heads[x], 0)
                    for st in range(ns):
                        for x in range(2):
                            if st + 1 < ns:
                                buf[(x, st + 1)] = emit_sc(heads[x], st + 1)
                        if st == 0:
                            while pending_mul:
                                pending_mul.pop(0)()
                        for x in range(2):
                            if st % 2 == 0:
                                PT_new = pt_pool.tile([P, 1024], fp8, tag="PT")
                                PTs[x] = PT_new
                            emit_exp(st, buf.pop((x, st)), PTs[x])
                        if st % 2 == 1:
                            pi = st // 2
                            for x in range(2):
                                nc.tensor.matmul(
                                    pos[x][: D + 1, :],
                                    V_aug[:, 2 * pi : 2 * pi + 2,
                                          heads[x], 0 : D + 1],
                                    PTs[x][:].rearrange(
                                        "p (two n) -> p two n", two=2),
                                    start=(pi == 0), stop=(pi == npairs - 1),
                                    perf_mode=DR,
                                )
                    # denominators for both heads (multiplies deferred into
                    # the next pair so they never block its exp chain)
                    for x in range(2):
                        h = heads[x]
                        hp = (h % 2) * D
                        hc = h // 2
                        po = pos[x]
                        dsum = small.tile([1, 512], f32, tag="dsum")
                        nc.scalar.activation(
                            out=dsum[:], in_=po[D : D + 1, :], func=AF.Identity)
                        rec = small.tile([1, 512], f32, tag="rec")
                        nc.vector.reciprocal_approx_fast(out=rec[:], in_=dsum[:])
                        den = denp.tile([D, 512], f32, tag="den")
                        nc.gpsimd.partition_broadcast(den[:], rec[:])

                        def ot_mul(po=po, den=den, hp=hp, hc=hc):
                            nc.vector.tensor_mul(
                                out=OT[hp : hp + D, hc, r * 512 : (r + 1) * 512],
                                in0=po[0:D, :],
                                in1=den[:],
                            )
                        pending_mul.append(ot_mul)

                    # drain filler tensor work (next run's V/QK)
                    npair_left = (HPC - h0) // 2
                    take = (len(fillers) + npair_left - 1) // npair_left
                    for _ in range(take):
                        fillers.pop(0)()

                while pending_mul:
                    pending_mul.pop(0)()

                # --- c_proj partial (+bproj/2) + AllReduce chunk ---
                for tt in range(4 * r, 4 * r + 4):
                    cc_sb = work.tile([P, C], bf16, tag="ccbuf")
                    for half in range(2):
                        ps = ppool.tile([P, 512], f32, tag="mm")
                        for ek in range(EK):
                            nc.tensor.matmul(
                                ps[:],
                                OT[:, ek, tt * P : (tt + 1) * P],
                                wp_sb[:, ek, half * 512 : (half + 1) * 512],
                                start=(ek == 0), stop=(ek == EK - 1),
                            )
                        nc.vector.tensor_copy(
                            out=cc_sb[:, half * 512 : (half + 1) * 512],
                            in_=ps[:],
                        )
                    nc.sync.dma_start(
                        cc_in_d[tt * P : (tt + 1) * P, :], cc_sb[:])

                nc.gpsimd.collective_compute(
                    "AllReduce",
                    ALU.add,
                    replica_groups=[[0, 1], [2, 3], [4, 5], [6, 7]],
                    ins=[cc_in_d[r * 512 : (r + 1) * 512, :].opt()],
                    outs=[cc_out_d[r * 512 : (r + 1) * 512, :].opt()],
                )
                if r == NR - 1:
                    emit_x2_run(0)
                    emit_x2_run(1)
                if r == NR - 2:
                    # run-3 QKV work is already emitted; free its inputs and
                    # prefetch the first fc weight chunks during run 3
                    xnTb_cm.__exit__(None, None, None)
                    xnT_cm.__exit__(None, None, None)
                    wearly_cm.__exit__(None, None, None)
                    wfe_cm = tc.tile_pool(name="wfearly", bufs=1, side="right")
                    wfe = wfe_cm.__enter__()
                    wfA = wfe.tile([P, 5, FPC], bf16)
                    wfT_r = wfT_d.ap().rearrange("(k p) o -> p k o", p=P)
                    for ck in range(5):
                        eng = nc.scalar if ck % 2 == 0 else nc.gpsimd
                        eng.dma_start(wfA[:, ck, :], wfT_r[:, ck, :])

            # release attention-phase SBUF before the MLP phase
            pt_cm.__exit__(None, None, None)
            wp_cm.__exit__(None, None, None)
            attn_cm.__exit__(None, None, None)

            with tc.tile_pool(name="wlate", bufs=1, side="right") as wlate, \
                 tc.tile_pool(name="p_hT", bufs=1, side="right") as p_hT:
                wfB = wlate.tile([P, CK - 5, FPC], bf16)
                wfT_r = wfT_d.ap().rearrange("(k p) o -> p k o", p=P)
                for ck in range(5, CK):
                    eng = nc.scalar if ck % 2 == 0 else nc.gpsimd
                    eng.dma_start(wfB[:, ck - 5, :], wfT_r[:, ck, :])
                wm_sb = wlate.tile([P, FK, C], bf16)
                wmT_r = wmT_d.ap().rearrange("(k p) o -> p k o", p=P)
                for fk in range(0, FK, 4):
                    eng = nc.scalar if (fk // 4) % 2 == 0 else nc.gpsimd
                    eng.dma_start(
                        wm_sb[:, fk : fk + 4, :], wmT_r[:, fk : fk + 4, :])

                # ======== MLP in 4 token quarters ========
                for tq in range(4):
                    if tq in (0, 1):
                        emit_x2_run(tq + 2)
                    t0 = tq * 512
                    hT = p_hT.tile([P, FK, 512], bf16, tag="hT")
                    for ft in range(FK):
                        ps = ppool.tile([P, 512], f32, tag="mm")
                        for ck in range(CK):
                            wsl = (wfA[:, ck, ft * P : (ft + 1) * P]
                                   if ck < 5 else
                                   wfB[:, ck - 5, ft * P : (ft + 1) * P])
                            nc.tensor.matmul(
                                ps[:],
                                wsl,
                                xn2T[:, ck, t0 : t0 + 512],
                                start=(ck == 0), stop=(ck == CK - 1),
                            )
                        nc.scalar.activation(
                            out=hT[:, ft, :], in_=ps[:],
                            func=AF.Gelu_apprx_tanh,
                            bias=bfc_sb[:, ft : ft + 1], scale=1.0,
                        )
                    for tl in range(4):
                        out_sb = work.tile([P, C], f32, tag="f32buf")
                        for half in range(2):
                            ps = ppool.tile([P, 512], f32, tag="mm")
                            for fk in range(FK):
                                nc.tensor.matmul(
                                    ps[:],
                                    hT[:, fk, tl * P : (tl + 1) * P],
                                    wm_sb[:, fk, half * 512 : (half + 1) * 512],
                                    start=(fk == 0), stop=(fk == FK - 1),
                                )
                            nc.vector.tensor_copy(
                                out=out_sb[:, half * 512 : (half + 1) * 512],
                                in_=ps[:],
                            )
                        nc.gpsimd.dma_start(
                            out_mlp_d[t0 + tl * P : t0 + (tl + 1) * P, :],
                            out_sb[:],
                        )

            wfe_cm.__exit__(None, None, None)
            xn2T_cm.__exit__(None, None, None)

    nc.finalize()
    return nc


def _prep_inputs(x, w_attn, b_attn, w_proj, b_proj, w_fc, b_fc, w_mlp_proj):
    bf = ml_dtypes.bfloat16
    f8 = ml_dtypes.float8_e4m3
    negmask = np.where(
        np.triu(np.ones((P, P), dtype=np.float32)) > 0, 0.0, -1e5
    ).astype(np.float32)

    # lhsT column permutation for Q/K tiles: tile = 2g+ktd, col m ->
    # row 64*(4g + m//32) + 32*ktd + m%32 of the local weight slice
    tiles = np.arange(4)
    m = np.arange(P)
    g = tiles // 2
    ktd = tiles % 2
    rows = (64 * (4 * g[:, None] + m[None, :] // 32)
            + 32 * ktd[:, None] + m[None, :] % 32)  # [4, 128]
    qk_rows = rows.reshape(-1)  # [512]

    def dr_pack(wl, permute):
        # wl [512 out, 1024 feat] -> [128p, 4ck, 2kt, 512 out] fp8
        # feature f = 256*ck + 2*p + kt
        if permute:
            wl = wl[qk_rows, :]
        w4 = wl.reshape(512, CKD, 2, P)          # [out, j, kt, p]
        return np.ascontiguousarray(w4.transpose(3, 1, 2, 0)).astype(f8)

    in_maps = []
    for core in range(8):
        b, s = divmod(core, 2)
        wq = w_attn[s * 512 : (s + 1) * 512, :]
        wk = w_attn[C + s * 512 : C + (s + 1) * 512, :]
        wv = w_attn[2 * C + s * 512 : 2 * C + (s + 1) * 512, :]
        bq = b_attn[s * 512 : (s + 1) * 512]
        bk = b_attn[C + s * 512 : C + (s + 1) * 512]
        bv = b_attn[2 * C + s * 512 : 2 * C + (s + 1) * 512]
        bqk = np.concatenate(
            [bq.reshape(EK, P).T, bk.reshape(EK, P).T], axis=1
        ).astype(np.float32)  # [128, 8] (4 Q tiles, 4 K tiles)
        wp = np.ascontiguousarray(w_proj[:, s * 512 : (s + 1) * 512].T).astype(bf)
        wf = np.ascontiguousarray(w_fc[s * FPC : (s + 1) * FPC, :].T).astype(bf)
        bfc = np.ascontiguousarray(
            b_fc[s * FPC : (s + 1) * FPC].reshape(FK, P).T).astype(np.float32)
        wm = np.ascontiguousarray(
            w_mlp_proj[:, s * FPC : (s + 1) * FPC].T).astype(bf)
        in_maps.append(
            {
                "x": np.ascontiguousarray(x[b]),
                "wq8": dr_pack(wq, False),
                "wk8": dr_pack(wk, False),
                "wv8": dr_pack(wv, False),
                "wpT": wp, "wfT": wf, "wmT": wm,
                "bqk": np.ascontiguousarray(bqk),
                "bv": np.ascontiguousarray(bv).astype(np.float32),
                "bprojh": (b_proj + w_proj @ b_attn[2 * C : 3 * C]).astype(bf),
                "bfc": bfc, "negmask": negmask,
            }
        )
    return in_maps


def run(x, w_attn, b_attn, w_proj, b_proj, w_fc, b_fc, w_mlp_proj, b_mlp_proj,
        trace=False):
    from concourse.bass_utils import run_bass_kernel_spmd

    if "nc" not in _CACHED:
        _CACHED["nc"] = _build_nc()
    nc = _CACHED["nc"]
    in_maps = _prep_inputs(
        x, w_attn, b_attn, w_proj, b_proj, w_fc, b_fc, w_mlp_proj
    )
    res = run_bass_kernel_spmd(
        nc, in_maps, core_ids=list(range(8)), trace=trace,
        trace_cores=list(range(8)) if trace else None,
    )
    out = np.empty((B, T, C), dtype=np.float32)
    for b in range(B):
        a = res.results[2 * b]
        c2 = res.results[2 * b + 1]
        out[b] = a["out_x2"] + a["out_mlp"] + c2["out_mlp"] + b_mlp_proj[None, :]
    return out, res


def kernel(x, w_attn, b_attn, w_proj, b_proj, w_fc, b_fc, w_mlp_proj, b_mlp_proj):
    out, _ = run(
        np.asarray(x, dtype=np.float32),
        np.asarray(w_attn, dtype=np.float32),
        np.asarray(b_attn, dtype=np.float32),
        np.asarray(w_proj, dtype=np.float32),
        np.asarray(b_proj, dtype=np.float32),
        np.asarray(w_fc, dtype=np.float32),
        np.asarray(b_fc, dtype=np.float32),
        np.asarray(w_mlp_proj, dtype=np.float32),
        np.asarray(b_mlp_proj, dtype=np.float32),
    )
    return out


# revision 32
# speedup vs baseline: 1.3286x; 1.0031x over previous
"""Trainium2 Bass kernel for a GPT-style transformer block.

Problem: nn_Block_36807869727037 (dense_transformer)
  B=4, T=2048, C=1024, H=16 heads (d=64), fp32 I/O.
  y = x + attn(LN1(x)); y = y + mlp(LN2(y))  (causal attention, tanh-GELU MLP)

Sharding (8 cores, one uniform SPMD program):
  core = 2*b + s  -> batch b in [0,4), tensor-parallel shard s in [0,2).
  Shard s owns heads [8s, 8s+8) and FFN hidden slice [2048s, 2048s+2048).
  Each core runs the full sequence (T=2048) for its batch:
    LN1 (duplicated in pair) -> QKV for its 8 heads -> causal attention ->
    c_proj partial (+b_proj/2) -> pairwise AllReduce (bf16) -> x2 = x + cc ->
    LN2 (duplicated) -> fc half + GELU -> mlp_proj partial.
  Final combine on host: out[b] = x2 (from core 2b) + mlp_partial(2b)
                                  + mlp_partial(2b+1) + b_mlp_proj.

Precision: QKV projection, scores (QK^T) and PV run in fp8e4 with
DoubleRow perf mode (2x PE throughput); c_proj and the MLP stay bf16.
PSUM accumulation is fp32 everywhere. Scores are computed transposed
S^T[k,q]; softmax uses no max-subtraction (|scores/8| < ~3); the causal
mask is applied additively on PSUM scores BEFORE exp; the denominator
comes from an appended ones-column in V and is applied via DVE
reciprocal + GpSimd partition-broadcast (no tensor-engine involvement).
LayerNorm rstd uses a DVE-only Newton rsqrt so the scalar engine keeps
the exp activation table resident for the whole attention phase.
"""

import sys

sys.path.insert(0, "/opt/trn_rl_repo")

import numpy as np
import ml_dtypes

B, T, C, H = 4, 2048, 1024, 16
D = C // H          # 64 head dim
HPC = H // 2        # 8 heads per core
FPC = 2 * C         # 2048 ffn hidden per core
P = 128
NT = T // P         # 16 token tiles
NR = T // 512       # 4 query runs of 512
CK = C // P         # 8 feature chunks (bf16 path)
CKD = C // 256      # 4 double-row feature chunks (fp8 path)
FK = FPC // P       # 16 ffn chunks per core
EK = (HPC * D) // P  # 4 head-dim chunks per core (512/128)
MAGIC = 0x5F3759DF

_CACHED = {}


def _build_nc():
    import concourse.bass as bass
    import concourse.mybir as mybir
    import concourse.tile as tile
    from concourse import bacc

    f32 = mybir.dt.float32
    bf16 = mybir.dt.bfloat16
    fp8 = mybir.dt.float8e4
    u16 = mybir.dt.uint16
    i32 = mybir.dt.int32
    AF = mybir.ActivationFunctionType
    ALU = mybir.AluOpType
    DR = mybir.MatmulPerfMode.DoubleRow

    nc = bacc.Bacc(trn_type="TRN2", target_bir_lowering=False, num_devices=8)

    # ---- I/O ----
    x_d = nc.dram_tensor("x", [T, C], f32, kind="ExternalInput")
    wq8_d = nc.dram_tensor("wq8", [P, CKD, 2, HPC * D], fp8, kind="ExternalInput")
    wk8_d = nc.dram_tensor("wk8", [P, CKD, 2, HPC * D], fp8, kind="ExternalInput")
    wv8_d = nc.dram_tensor("wv8", [P, CKD, 2, HPC * D], fp8, kind="ExternalInput")
    wpT_d = nc.dram_tensor("wpT", [HPC * D, C], bf16, kind="ExternalInput")
    wfT_d = nc.dram_tensor("wfT", [C, FPC], bf16, kind="ExternalInput")
    wmT_d = nc.dram_tensor("wmT", [FPC, C], bf16, kind="ExternalInput")
    bqk_d = nc.dram_tensor("bqk", [P, 8], f32, kind="ExternalInput")
    bv_d = nc.dram_tensor("bv", [HPC * D], f32, kind="ExternalInput")
    bprojh_d = nc.dram_tensor("bprojh", [C], bf16, kind="ExternalInput")
    bfc_d = nc.dram_tensor("bfc", [P, FK], f32, kind="ExternalInput")
    negmask_d = nc.dram_tensor("negmask", [P, P], f32, kind="ExternalInput")

    out_mlp_d = nc.dram_tensor("out_mlp", [T, C], f32, kind="ExternalOutput")
    out_x2_d = nc.dram_tensor("out_x2", [T, C], f32, kind="ExternalOutput")

    cc_in_d = nc.dram_tensor("cc_in", [T, C], bf16)
    cc_out_d = nc.dram_tensor("cc_out", [T, C], bf16)

    def bcast_row(dram_ap, n):
        return bass.AP(
            tensor=dram_ap.tensor, offset=dram_ap.offset,
            ap=[[0, P], *dram_ap.ap],
        )

    with tile.TileContext(nc, pool_alloc_mode="queue") as tc:
        import contextlib

        with contextlib.ExitStack() as ctx:
            consts = ctx.enter_context(tc.tile_pool(name="consts", bufs=1))
            work = ctx.enter_context(tc.tile_pool(name="work", bufs=2))
            xpool = ctx.enter_context(tc.tile_pool(name="xpool", bufs=1))
            ln_pool = ctx.enter_context(tc.tile_pool(name="ln", bufs=2))
            small = ctx.enter_context(tc.tile_pool(name="small", bufs=1))
            denp = ctx.enter_context(tc.tile_pool(name="denp", bufs=2))
            ppool = ctx.enter_context(tc.tile_pool(name="psum", bufs=2, space="PSUM"))
            scpool = ctx.enter_context(
                tc.tile_pool(name="psum_sc", bufs=4, space="PSUM"))
            pvpool = ctx.enter_context(
                tc.tile_pool(name="psum_pv", bufs=1, space="PSUM"))

            # ---- constants ----
            negmask_sb = consts.tile([P, P], f32)
            nc.scalar.dma_start(negmask_sb[:], negmask_d[:])
            bqk_sb = consts.tile([P, 8], f32)
            nc.scalar.dma_start(bqk_sb[:], bqk_d[:])
            bfc_sb = consts.tile([P, FK], f32)
            nc.scalar.dma_start(bfc_sb[:], bfc_d[:])
            bproj_sb = consts.tile([P, C], bf16)
            nc.scalar.dma_start(bproj_sb[:], bcast_row(bprojh_d[:], C))
            magic_sb = consts.tile([P, 4], i32)
            nc.vector.memset(magic_sb[:], MAGIC)
            dummy_sb = consts.tile([P, 1], f32)
            nc.vector.memset(dummy_sb[:], 0.0)
            # preload the exp activation table while startup DMAs run
            nc.scalar.activation(
                out=dummy_sb[:], in_=dummy_sb[:], func=AF.Exp, scale=1.0)

            def rsqrt_newton(v_ap, n, tag):
                # v_ap: [P, n] fp32 (possibly strided); returns [P, n] ~1/sqrt(v)
                y = ln_pool.tile([P, n], f32, tag=tag + "_y")
                t = ln_pool.tile([P, n], f32, tag=tag + "_t")
                yb = y[:].bitcast(i32)
                nc.vector.tensor_scalar(
                    out=yb, in0=v_ap.bitcast(i32), scalar1=1, scalar2=None,
                    op0=ALU.logical_shift_right,
                )
                nc.vector.tensor_tensor(
                    out=yb, in0=magic_sb[:, :n], in1=yb, op=ALU.subtract)
                for _ in range(2):
                    nc.vector.tensor_tensor(
                        out=t[:], in0=y[:], in1=y[:], op=ALU.mult)
                    nc.vector.tensor_tensor(
                        out=t[:], in0=t[:], in1=v_ap, op=ALU.mult)
                    nc.vector.tensor_scalar(
                        out=t[:], in0=t[:], scalar1=-0.5, scalar2=1.5,
                        op0=ALU.mult, op1=ALU.add,
                    )
                    nc.vector.tensor_tensor(
                        out=y[:], in0=y[:], in1=t[:], op=ALU.mult)
                return y

            # persistent activation tensors (released before MLP)
            attn_cm = tc.tile_pool(name="attn", bufs=1)
            attn_pool = attn_cm.__enter__()
            QT = attn_pool.tile([P, EK, T], bf16)
            KT = attn_pool.tile([P, EK, T], bf16)
            V_aug = attn_pool.tile([P, NT, HPC, 72], fp8)
            OT = attn_pool.tile([P, EK, T], bf16)
            nc.vector.memset(V_aug[:, :, :, D : D + 1], 1.0)

            wp_cm = tc.tile_pool(name="wp", bufs=1)
            wp_pool = wp_cm.__enter__()
            wp_sb = wp_pool.tile([P, EK, C], bf16)
            nc.scalar.dma_start(
                wp_sb[:], wpT_d.ap().rearrange("(k p) o -> p k o", p=P))

            pt_cm = tc.tile_pool(name="ptp", bufs=4)
            pt_pool = pt_cm.__enter__()

            wearly_cm = tc.tile_pool(name="wearly", bufs=1)
            wearly = wearly_cm.__enter__()
            wq_sb = wearly.tile([P, CKD, 2, HPC * D], fp8)
            wk_sb = wearly.tile([P, CKD, 2, HPC * D], fp8)
            wv_sb = wearly.tile([P, CKD, 2, HPC * D], fp8)
            nc.gpsimd.dma_start(wv_sb[:], wv8_d.ap())
            nc.scalar.dma_start(wq_sb[:], wq8_d.ap())
            nc.scalar.dma_start(wk_sb[:], wk8_d.ap())
            xnT_cm = tc.tile_pool(name="p_xnT", bufs=1)
            p_xnT = xnT_cm.__enter__()
            xnTb_cm = tc.tile_pool(name="p_xnTb", bufs=1)
            p_xnTb = xnTb_cm.__enter__()
            # normalized x: bf16 transposed per-run buffer (f = 128ck + p),
            # cast to fp8 xnT8; DoubleRow k-tiles are chunk pairs (2j, 2j+1)
            xnT8 = p_xnT.tile([P, CK, T], fp8)


            xn2T_cm = tc.tile_pool(name="p_xn2T", bufs=1, side="right")
            p_xn2T = xn2T_cm.__enter__()
            xn2T = p_xn2T.tile([P, CK, T], bf16)

            def xnT8_dr(j, t0, n):
                # [P, 2, n] fp8 chunk-pair view for DoubleRow matmuls
                return xnT8[:, 2 * j : 2 * j + 2, t0 : t0 + n]

            def emit_ln1_dma(rr):
                xr = xpool.tile([P, 4, C], f32, tag="xres")
                nc.sync.dma_start(
                    xr[:],
                    x_d[rr * 512 : (rr + 1) * 512, :].rearrange(
                        "(t p) c -> p t c", p=P),
                )
                xnTb = p_xnTb.tile([P, CK, 512], bf16, tag="xnTb")
                return xr, xnTb

            def emit_ln1_chunk(rr, half, xr, xnTb):
                # LN1 for 2 tiles of run rr -> xnT8 (fp8, transposed)
                mvb = ln_pool.tile([P, 2, 2], f32, tag="ln1_mv")
                for i2 in range(2):
                    i = 2 * half + i2
                    xg = xr[:, i, :].rearrange("p (g f) -> p g f", f=512)
                    stats = ln_pool.tile([P, 2, 6], f32, tag="ln1_st")
                    for g in range(2):
                        nc.vector.bn_stats(out=stats[:, g, :], in_=xg[:, g, :])
                    nc.vector.bn_aggr(out=mvb[:, i2, :], in_=stats[:])
                rstd = rsqrt_newton(mvb[:, :, 1], 2, "ln1")
                for i2 in range(2):
                    i = 2 * half + i2
                    tt = 4 * rr + i
                    xn_bf = work.tile([P, C], bf16, tag="xn8")
                    nc.vector.tensor_scalar(
                        out=xn_bf[:], in0=xr[:, i, :],
                        scalar1=mvb[:, i2, 0:1], scalar2=rstd[:, i2 : i2 + 1],
                        op0=ALU.subtract, op1=ALU.mult,
                    )
                    nc.sync.dma_start_transpose(
                        xnTb[:, :, i * P : (i + 1) * P], xn_bf[:])
                    nc.vector.tensor_copy(
                        out=xnT8[:, :, tt * P : (tt + 1) * P],
                        in_=xnTb[:, :, i * P : (i + 1) * P],
                    )

            def emit_ln1_run(rr):
                xr, xnTb = emit_ln1_dma(rr)
                emit_ln1_chunk(rr, 0, xr, xnTb)
                emit_ln1_chunk(rr, 1, xr, xnTb)

            def emit_v_tile(tt):
                ps = ppool.tile([P, 512], f32, tag="mm")
                for ck in range(CKD):
                    nc.tensor.matmul(
                        ps[:],
                        xnT8_dr(ck, tt * P, P),
                        wv_sb[:, ck, :, :],
                        start=(ck == 0), stop=(ck == CKD - 1),
                        perf_mode=DR,
                    )
                nc.vector.tensor_copy(
                    out=V_aug[:, tt, :, 0:D],
                    in_=ps[:].rearrange("p (h e) -> p h e", h=HPC),
                )

            def emit_qk_tile(r, ot):
                # ot in 0..8: 0-3 Q tiles, 4-7 K tiles (natural 128-col chunks)
                w_sb = wq_sb if ot < 4 else wk_sb
                dst = QT if ot < 4 else KT
                ti = ot % 4
                ps = ppool.tile([P, 512], f32, tag="mm")
                for ck in range(CKD):
                    nc.tensor.matmul(
                        ps[:],
                        w_sb[:, ck, :, ti * P : (ti + 1) * P],
                        xnT8_dr(ck, r * 512, 512),
                        start=(ck == 0), stop=(ck == CKD - 1),
                        perf_mode=DR,
                    )
                nc.vector.tensor_scalar(
                    out=dst[:, ti, r * 512 : (r + 1) * 512],
                    in0=ps[:], scalar1=bqk_sb[:, ot : ot + 1], scalar2=None,
                    op0=ALU.add,
                )

            def emit_x2_run(rr):
                # x2 = x + cc (attn partial sum incl b_proj); LN2; transpose
                mvb = ln_pool.tile([P, 4, 2], f32, tag="ln2_mv")
                xr = xpool.tile([P, 4, C], f32, tag="xres")
                dslice = x_d[rr * 512 : (rr + 1) * 512, :].rearrange(
                    "(t p) c -> p t c", p=P)
                nc.sync.dma_start(xr[:], dslice)
                att_sb = xpool.tile([P, 4, C], bf16, tag="attres")
                nc.sync.dma_start(
                    att_sb[:],
                    cc_out_d[rr * 512 : (rr + 1) * 512, :].rearrange(
                        "(t p) c -> p t c", p=P),
                )
                nc.vector.tensor_add(out=xr[:], in0=xr[:], in1=att_sb[:])
                nc.sync.dma_start(
                    out_x2_d[rr * 512 : (rr + 1) * 512, :].rearrange(
                        "(t p) c -> p t c", p=P),
                    xr[:],
                )
                x2s = []
                for i in range(4):
                    x_sb = xr[:, i, :]
                    xg = x_sb.rearrange("p (g f) -> p g f", f=512)
                    stats = ln_pool.tile([P, 2, 6], f32, tag="ln2_st")
                    for g in range(2):
                        nc.vector.bn_stats(out=stats[:, g, :], in_=xg[:, g, :])
                    nc.vector.bn_aggr(out=mvb[:, i, :], in_=stats[:])
                    x2s.append(x_sb)
                rstd = rsqrt_newton(mvb[:, :, 1], 4, "ln2")
                for i in range(4):
                    tt = 4 * rr + i
                    xn2_bf = work.tile([P, C], bf16, tag="xn2bf")
                    nc.vector.tensor_scalar(
                        out=xn2_bf[:], in0=x2s[i],
                        scalar1=mvb[:, i, 0:1], scalar2=rstd[:, i : i + 1],
                        op0=ALU.subtract, op1=ALU.mult,
                    )
                    nc.sync.dma_start_transpose(
                        xn2T[:, :, tt * P : (tt + 1) * P], xn2_bf[:])

            # ======== fused pipeline over the 4 token runs ========
            for r in range(NR):
                if r == 0:
                    xr0, xnTb0 = emit_ln1_dma(0)
                    emit_ln1_chunk(0, 0, xr0, xnTb0)
                    emit_v_tile(0)
                    emit_v_tile(1)
                    emit_ln1_chunk(0, 1, xr0, xnTb0)
                    emit_v_tile(2)
                    emit_v_tile(3)
                    for ot in range(8):
                        emit_qk_tile(0, ot)
                fillers = []
                if r < NR - 1:
                    emit_ln1_run(r + 1)
                    for tt in range(4 * (r + 1), 4 * (r + 1) + 4):
                        fillers.append(lambda tt=tt: emit_v_tile(tt))
                    for ot in range(8):
                        fillers.append(lambda ot=ot: emit_qk_tile(r + 1, ot))

                # --- attention: heads processed in interleaved pairs so the
                # tensor engine always has independent work while exp runs ---
                ns = 4 * r + 4
                npairs = ns // 2
                pending_mul = []

                def emit_sc(h, st):
                    hp = (h % 2) * D
                    hc = h // 2
                    sc = scpool.tile([P, 512], f32, tag="sc")
                    nc.tensor.matmul(
                        sc[:],
                        KT[hp : hp + D, hc, st * P : (st + 1) * P],
                        QT[hp : hp + D, hc, r * 512 : (r + 1) * 512],
                        start=True, stop=True,
                    )
                    return sc

                def emit_exp(st, sc, PT):
                    j = st - 4 * r
                    off = (st % 2) * 512
                    if j < 0:
                        nc.scalar.activation(
                            out=PT[:, off : off + 512], in_=sc[:],
                            func=AF.Exp, scale=0.125)
                    else:
                        nc.vector.tensor_add(
                            out=sc[:, j * P : (j + 1) * P],
                            in0=sc[:, j * P : (j + 1) * P],
                            in1=negmask_sb[:],
                        )
                        nc.scalar.activation(
                            out=PT[:, off + j * P : off + 512],
                            in_=sc[:, j * P : 512],
                            func=AF.Exp, scale=0.125)
                        if j > 0:
                            nc.gpsimd.memset(PT[:, off : off + j * P], 0.0)

                for h0 in range(0, HPC, 2):
                    heads = (h0, h0 + 1)
                    po_a = pvpool.tile([P, 512], f32, tag="pv0")
                    po_b = pvpool.tile([P, 512], f32, tag="pv1")
                    pos = [po_a, po_b]
                    PTs = [None, None]
                    buf = {}
                    for x in range(2):
                        buf[(x, 0)] = emit_sc(heads[x], 0)
                    for st in range(ns):
                        for x in range(2):
                            if st + 1 < ns:
                                buf[(x, st + 1)] = emit_sc(heads[x], st + 1)
                        if st == 0:
                            while pending_mul:
                                pending_mul.pop(0)()
                        for x in range(2):
                            if st % 2 == 0:
                                PT_new = pt_pool.tile([P, 1024], fp8, tag="PT")
                                PTs[x] = PT_new
                            emit_exp(st, buf.pop((x, st)), PTs[x])
                        if st % 2 == 1:
                            pi = st // 2
                            for x in range(2):
                                nc.tensor.matmul(
                                    pos[x][: D + 1, :],
                                    V_aug[:, 2 * pi : 2 * pi + 2,
                                          heads[x], 0 : D + 1],
                                    PTs[x][:].rearrange(
                                        "p (two n) -> p two n", two=2),
                                    start=(pi == 0), stop=(pi == npairs - 1),
                                    perf_mode=DR,
                                )
                    # denominators for both heads (multiplies deferred into
                    # the next pair so they never block its exp chain)
                    for x in range(2):
                        h = heads[x]
                        hp = (h % 2) * D
                        hc = h // 2
                        po = pos[x]
                        dsum = small.tile([1, 512], f32, tag="dsum")
                        nc.scalar.activation(
                            out=dsum[:], in_=po[D : D + 1, :], func=AF.Identity)
                        rec = small.tile([1, 512], f32, tag="rec")
                        nc.vector.reciprocal_approx_fast(out=rec[:], in_=dsum[:])
                        den = denp.tile([D, 512], f32, tag="den")
                        nc.gpsimd.partition_broadcast(den[:], rec[:])

                        def ot_mul(po=po, den=den, hp=hp, hc=hc):
                            nc.vector.tensor_mul(
                                out=OT[hp : hp + D, hc, r * 512 : (r + 1) * 512],
                                in0=po[0:D, :],
                                in1=den[:],
                            )
                        pending_mul.append(ot_mul)

                    # drain filler tensor work (next run's V/QK)
                    npair_left = (HPC - h0) // 2
                    take = (len(fillers) + npair_left - 1) // npair_left
                    for _ in range(take):
                        fillers.pop(0)()

                while pending_mul:
                    pending_mul.pop(0)()

                # --- c_proj partial (+bproj/2) + AllReduce chunk ---
                for tt in range(4 * r, 4 * r + 4):
                    cc_sb = work.tile([P, C], bf16, tag="ccbuf")
                    for half in range(2):
                        ps = ppool.tile([P, 512], f32, tag="mm")
                        for ek in range(EK):
                            nc.tensor.matmul(
                                ps[:],
                                OT[:, ek, tt * P : (tt + 1) * P],
                                wp_sb[:, ek, half * 512 : (half + 1) * 512],
                                start=(ek == 0), stop=(ek == EK - 1),
                            )
                        nc.vector.tensor_add(
                            out=cc_sb[:, half * 512 : (half + 1) * 512],
                            in0=ps[:],
                            in1=bproj_sb[:, half * 512 : (half + 1) * 512],
                        )
                    nc.sync.dma_start(
                        cc_in_d[tt * P : (tt + 1) * P, :], cc_sb[:])

                nc.gpsimd.collective_compute(
                    "AllReduce",
                    ALU.add,
                    replica_groups=[[0, 1], [2, 3], [4, 5], [6, 7]],
                    ins=[cc_in_d[r * 512 : (r + 1) * 512, :].opt()],
                    outs=[cc_out_d[r * 512 : (r + 1) * 512, :].opt()],
                )
                if r == NR - 1:
                    emit_x2_run(0)
                    emit_x2_run(1)
                if r == NR - 2:
                    # run-3 QKV work is already emitted; free its inputs and
                    # prefetch the first fc weight chunks during run 3
                    xnTb_cm.__exit__(None, None, None)
                    xnT_cm.__exit__(None, None, None)
                    wearly_cm.__exit__(None, None, None)
                    wfe_cm = tc.tile_pool(name="wfearly", bufs=1, side="right")
                    wfe = wfe_cm.__enter__()
                    wfA = wfe.tile([P, 5, FPC], bf16)
                    wfT_r = wfT_d.ap().rearrange("(k p) o -> p k o", p=P)
                    for ck in range(5):
                        eng = nc.scalar if ck % 2 == 0 else nc.gpsimd
                        eng.dma_start(wfA[:, ck, :], wfT_r[:, ck, :])

            # release attention-phase SBUF before the MLP phase
            pt_cm.__exit__(None, None, None)
            wp_cm.__exit__(None, None, None)
            attn_cm.__exit__(None, None, None)

            with tc.tile_pool(name="wlate", bufs=1, side="right") as wlate, \
                 tc.tile_pool(name="p_hT", bufs=1, side="right") as p_hT:
                wfB = wlate.tile([P, CK - 5, FPC], bf16)
                wfT_r = wfT_d.ap().rearrange("(k p) o -> p k o", p=P)
                for ck in range(5, CK):
                    eng = nc.scalar if ck % 2 == 0 else nc.gpsimd
                    eng.dma_start(wfB[:, ck - 5, :], wfT_r[:, ck, :])
                wm_sb = wlate.tile([P, FK, C], bf16)
                wmT_r = wmT_d.ap().rearrange("(k p) o -> p k o", p=P)
                for fk in range(0, FK, 4):
                    eng = nc.scalar if (fk // 4) % 2 == 0 else nc.gpsimd
                    eng.dma_start(
                        wm_sb[:, fk : fk + 4, :], wmT_r[:, fk : fk + 4, :])

                # ======== MLP in 4 token quarters ========
                for tq in range(4):
                    if tq in (0, 1):
                        emit_x2_run(tq + 2)
                    t0 = tq * 512
                    hT = p_hT.tile([P, FK, 512], bf16, tag="hT")
                    for ft in range(FK):
                        ps = ppool.tile([P, 512], f32, tag="mm")
                        for ck in range(CK):
                            wsl = (wfA[:, ck, ft * P : (ft + 1) * P]
                                   if ck < 5 else
                                   wfB[:, ck - 5, ft * P : (ft + 1) * P])
                            nc.tensor.matmul(
                                ps[:],
                                wsl,
                                xn2T[:, ck, t0 : t0 + 512],
                                start=(ck == 0), stop=(ck == CK - 1),
                            )
                        nc.scalar.activation(
                            out=hT[:, ft, :], in_=ps[:],
                            func=AF.Gelu_apprx_tanh,
                            bias=bfc_sb[:, ft : ft + 1], scale=1.0,
                        )
                    for tl in range(4):
                        out_sb = work.tile([P, C], f32, tag="f32buf")
                        for half in range(2):
                            ps = ppool.tile([P, 512], f32, tag="mm")
                            for fk in range(FK):
                                nc.tensor.matmul(
                                    ps[:],
                                    hT[:, fk, tl * P : (tl + 1) * P],
                                    wm_sb[:, fk, half * 512 : (half + 1) * 512],
                                    start=(fk == 0), stop=(fk == FK - 1),
                                )
                            nc.vector.tensor_copy(
                                out=out_sb[:, half * 512 : (half + 1) * 512],
                                in_=ps[:],
                            )
                        nc.gpsimd.dma_start(
                            out_mlp_d[t0 + tl * P : t0 + (tl + 1) * P, :],
                            out_sb[:],
                        )

            wfe_cm.__exit__(None, None, None)
            xn2T_cm.__exit__(None, None, None)

    nc.finalize()
    return nc


def _prep_inputs(x, w_attn, b_attn, w_proj, b_proj, w_fc, b_fc, w_mlp_proj):
    bf = ml_dtypes.bfloat16
    f8 = ml_dtypes.float8_e4m3
    negmask = np.where(
        np.triu(np.ones((P, P), dtype=np.float32)) > 0, 0.0, -1e5
    ).astype(np.float32)

    # lhsT column permutation for Q/K tiles: tile = 2g+ktd, col m ->
    # row 64*(4g + m//32) + 32*ktd + m%32 of the local weight slice
    tiles = np.arange(4)
    m = np.arange(P)
    g = tiles // 2
    ktd = tiles % 2
    rows = (64 * (4 * g[:, None] + m[None, :] // 32)
            + 32 * ktd[:, None] + m[None, :] % 32)  # [4, 128]
    qk_rows = rows.reshape(-1)  # [512]

    def dr_pack(wl, permute):
        # wl [512 out, 1024 feat] -> [128p, 4ck, 2kt, 512 out] fp8
        # feature f = 256*ck + 2*p + kt
        if permute:
            wl = wl[qk_rows, :]
        w4 = wl.reshape(512, CKD, 2, P)          # [out, j, kt, p]
        return np.ascontiguousarray(w4.transpose(3, 1, 2, 0)).astype(f8)

    in_maps = []
    for core in range(8):
        b, s = divmod(core, 2)
        wq = w_attn[s * 512 : (s + 1) * 512, :]
        wk = w_attn[C + s * 512 : C + (s + 1) * 512, :]
        wv = w_attn[2 * C + s * 512 : 2 * C + (s + 1) * 512, :]
        bq = b_attn[s * 512 : (s + 1) * 512]
        bk = b_attn[C + s * 512 : C + (s + 1) * 512]
        bv = b_attn[2 * C + s * 512 : 2 * C + (s + 1) * 512]
        bqk = np.concatenate(
            [bq.reshape(EK, P).T, bk.reshape(EK, P).T], axis=1
        ).astype(np.float32)  # [128, 8] (4 Q tiles, 4 K tiles)
        wp = np.ascontiguousarray(w_proj[:, s * 512 : (s + 1) * 512].T).astype(bf)
        wf = np.ascontiguousarray(w_fc[s * FPC : (s + 1) * FPC, :].T).astype(bf)
        bfc = np.ascontiguousarray(
            b_fc[s * FPC : (s + 1) * FPC].reshape(FK, P).T).astype(np.float32)
        wm = np.ascontiguousarray(
            w_mlp_proj[:, s * FPC : (s + 1) * FPC].T).astype(bf)
        in_maps.append(
            {
                "x": np.ascontiguousarray(x[b]),
                "wq8": dr_pack(wq, False),
                "wk8": dr_pack(wk, False),
                "wv8": dr_pack(wv, False),
                "wpT": wp, "wfT": wf, "wmT": wm,
                "bqk": np.ascontiguousarray(bqk),
                "bv": np.ascontiguousarray(bv).astype(np.float32),
                "bprojh": ((b_proj + w_proj @ b_attn[2 * C : 3 * C]) if s == 0
                           else np.zeros(C, np.float32)).astype(bf),
                "bfc": bfc, "negmask": negmask,
            }
        )
    return in_maps


def run(x, w_attn, b_attn, w_proj, b_proj, w_fc, b_fc, w_mlp_proj, b_mlp_proj,
        trace=False):
    from concourse.bass_utils import run_bass_kernel_spmd

    if "nc" not in _CACHED:
        _CACHED["nc"] = _build_nc()
    nc = _CACHED["nc"]
    in_maps = _prep_inputs(
        x, w_attn, b_attn, w_proj, b_proj, w_fc, b_fc, w_mlp_proj
    )
    res = run_bass_kernel_spmd(
        nc, in_maps, core_ids=list(range(8)), trace=trace,
        trace_cores=list(range(8)) if trace else None,
    )
    out = np.empty((B, T, C), dtype=np.float32)
    for b in range(B):
        a = res.results[2 * b]
        c2 = res.results[2 * b + 1]
        out[b] = a["out_x2"] + a["out_mlp"] + c2["out_mlp"] + b_mlp_proj[None, :]
    return out, res


def kernel(x, w_attn, b_attn, w_proj, b_proj, w_fc, b_fc, w_mlp_proj, b_mlp_proj):
    out, _ = run(
        np.asarray(x, dtype=np.float32),
        np.asarray(w_attn, dtype=np.float32),
        np.asarray(b_attn, dtype=np.float32),
        np.asarray(w_proj, dtype=np.float32),
        np.asarray(b_proj, dtype=np.float32),
        np.asarray(w_fc, dtype=np.float32),
        np.asarray(b_fc, dtype=np.float32),
        np.asarray(w_mlp_proj, dtype=np.float32),
        np.asarray(b_mlp_proj, dtype=np.float32),
    )
    return out


# revision 33
# speedup vs baseline: 1.3525x; 1.0180x over previous
"""Trainium2 Bass kernel for a GPT-style transformer block.

Problem: nn_Block_36807869727037 (dense_transformer)
  B=4, T=2048, C=1024, H=16 heads (d=64), fp32 I/O.
  y = x + attn(LN1(x)); y = y + mlp(LN2(y))  (causal attention, tanh-GELU MLP)

Sharding (8 cores, one uniform SPMD program):
  core = 2*b + s  -> batch b in [0,4), tensor-parallel shard s in [0,2).
  Shard s owns heads [8s, 8s+8) and FFN hidden slice [2048s, 2048s+2048).
  Each core runs the full sequence (T=2048) for its batch:
    LN1 (duplicated in pair) -> QKV for its 8 heads -> causal attention ->
    c_proj partial (+b_proj/2) -> pairwise AllReduce (bf16) -> x2 = x + cc ->
    LN2 (duplicated) -> fc half + GELU -> mlp_proj partial.
  Final combine on host: out[b] = x2 (from core 2b) + mlp_partial(2b)
                                  + mlp_partial(2b+1) + b_mlp_proj.

Precision: QKV projection, scores (QK^T) and PV run in fp8e4 with
DoubleRow perf mode (2x PE throughput); c_proj and the MLP stay bf16.
PSUM accumulation is fp32 everywhere. Scores are computed transposed
S^T[k,q]; softmax uses no max-subtraction (|scores/8| < ~3); the causal
mask is applied additively on PSUM scores BEFORE exp; the denominator
comes from an appended ones-column in V and is applied via DVE
reciprocal + GpSimd partition-broadcast (no tensor-engine involvement).
LayerNorm rstd uses a DVE-only Newton rsqrt so the scalar engine keeps
the exp activation table resident for the whole attention phase.
"""

import sys

sys.path.insert(0, "/opt/trn_rl_repo")

import numpy as np
import ml_dtypes

B, T, C, H = 4, 2048, 1024, 16
D = C // H          # 64 head dim
HPC = H // 2        # 8 heads per core
FPC = 2 * C         # 2048 ffn hidden per core
P = 128
NT = T // P         # 16 token tiles
NR = T // 512       # 4 query runs of 512
CK = C // P         # 8 feature chunks (bf16 path)
CKD = C // 256      # 4 double-row feature chunks (fp8 path)
FK = FPC // P       # 16 ffn chunks per core
EK = (HPC * D) // P  # 4 head-dim chunks per core (512/128)
MAGIC = 0x5F3759DF

_CACHED = {}


def _build_nc():
    import concourse.bass as bass
    import concourse.mybir as mybir
    import concourse.tile as tile
    from concourse import bacc

    f32 = mybir.dt.float32
    bf16 = mybir.dt.bfloat16
    fp8 = mybir.dt.float8e4
    u16 = mybir.dt.uint16
    i32 = mybir.dt.int32
    AF = mybir.ActivationFunctionType
    ALU = mybir.AluOpType
    DR = mybir.MatmulPerfMode.DoubleRow

    nc = bacc.Bacc(trn_type="TRN2", target_bir_lowering=False, num_devices=8)

    # ---- I/O ----
    x_d = nc.dram_tensor("x", [T, C], f32, kind="ExternalInput")
    wq8_d = nc.dram_tensor("wq8", [P, CKD, 2, HPC * D], fp8, kind="ExternalInput")
    wk8_d = nc.dram_tensor("wk8", [P, CKD, 2, HPC * D], fp8, kind="ExternalInput")
    wv8_d = nc.dram_tensor("wv8", [P, CKD, 2, HPC * D], fp8, kind="ExternalInput")
    wpT_d = nc.dram_tensor("wpT", [HPC * D, C], bf16, kind="ExternalInput")
    wfT_d = nc.dram_tensor("wfT", [C, FPC], bf16, kind="ExternalInput")
    wmT_d = nc.dram_tensor("wmT", [FPC, C], bf16, kind="ExternalInput")
    bqk_d = nc.dram_tensor("bqk", [P, 8], f32, kind="ExternalInput")
    bv_d = nc.dram_tensor("bv", [HPC * D], f32, kind="ExternalInput")
    bprojh_d = nc.dram_tensor("bprojh", [C], bf16, kind="ExternalInput")
    bfc_d = nc.dram_tensor("bfc", [P, FK], f32, kind="ExternalInput")
    negmask_d = nc.dram_tensor("negmask", [P, P], f32, kind="ExternalInput")

    out_mlp_d = nc.dram_tensor("out_mlp", [T, C], f32, kind="ExternalOutput")
    out_x2_d = nc.dram_tensor("out_x2", [T, C], f32, kind="ExternalOutput")

    cc_in_d = nc.dram_tensor("cc_in", [T, C], bf16)
    cc_out_d = nc.dram_tensor("cc_out", [T, C], bf16)

    def bcast_row(dram_ap, n):
        return bass.AP(
            tensor=dram_ap.tensor, offset=dram_ap.offset,
            ap=[[0, P], *dram_ap.ap],
        )

    with tile.TileContext(nc, pool_alloc_mode="queue") as tc:
        import contextlib

        with contextlib.ExitStack() as ctx:
            consts = ctx.enter_context(tc.tile_pool(name="consts", bufs=1))
            work = ctx.enter_context(tc.tile_pool(name="work", bufs=2))
            xpool = ctx.enter_context(tc.tile_pool(name="xpool", bufs=1))
            ln_pool = ctx.enter_context(tc.tile_pool(name="ln", bufs=2))
            small = ctx.enter_context(tc.tile_pool(name="small", bufs=1))
            denp = ctx.enter_context(tc.tile_pool(name="denp", bufs=2))
            ppool = ctx.enter_context(tc.tile_pool(name="psum", bufs=2, space="PSUM"))
            scpool = ctx.enter_context(
                tc.tile_pool(name="psum_sc", bufs=4, space="PSUM"))
            pvpool = ctx.enter_context(
                tc.tile_pool(name="psum_pv", bufs=1, space="PSUM"))

            # ---- constants ----
            negmask_sb = consts.tile([P, P], f32)
            nc.scalar.dma_start(negmask_sb[:], negmask_d[:])
            bqk_sb = consts.tile([P, 8], f32)
            nc.scalar.dma_start(bqk_sb[:], bqk_d[:])
            bfc_sb = consts.tile([P, FK], f32)
            nc.scalar.dma_start(bfc_sb[:], bfc_d[:])
            bproj_sb = consts.tile([P, C], bf16)
            nc.scalar.dma_start(bproj_sb[:], bcast_row(bprojh_d[:], C))
            magic_sb = consts.tile([P, 4], i32)
            nc.vector.memset(magic_sb[:], MAGIC)
            dummy_sb = consts.tile([P, 1], f32)
            nc.vector.memset(dummy_sb[:], 0.0)
            # preload the exp activation table while startup DMAs run
            nc.scalar.activation(
                out=dummy_sb[:], in_=dummy_sb[:], func=AF.Exp, scale=1.0)

            def rsqrt_newton(v_ap, n, tag):
                # v_ap: [P, n] fp32 (possibly strided); returns [P, n] ~1/sqrt(v)
                y = ln_pool.tile([P, n], f32, tag=tag + "_y")
                t = ln_pool.tile([P, n], f32, tag=tag + "_t")
                yb = y[:].bitcast(i32)
                nc.vector.tensor_scalar(
                    out=yb, in0=v_ap.bitcast(i32), scalar1=1, scalar2=None,
                    op0=ALU.logical_shift_right,
                )
                nc.vector.tensor_tensor(
                    out=yb, in0=magic_sb[:, :n], in1=yb, op=ALU.subtract)
                for _ in range(2):
                    nc.vector.tensor_tensor(
                        out=t[:], in0=y[:], in1=y[:], op=ALU.mult)
                    nc.vector.tensor_tensor(
                        out=t[:], in0=t[:], in1=v_ap, op=ALU.mult)
                    nc.vector.tensor_scalar(
                        out=t[:], in0=t[:], scalar1=-0.5, scalar2=1.5,
                        op0=ALU.mult, op1=ALU.add,
                    )
                    nc.vector.tensor_tensor(
                        out=y[:], in0=y[:], in1=t[:], op=ALU.mult)
                return y

            # persistent activation tensors (released before MLP)
            attn_cm = tc.tile_pool(name="attn", bufs=1)
            attn_pool = attn_cm.__enter__()
            QT = attn_pool.tile([P, EK, T], bf16)
            KT = attn_pool.tile([P, EK, T], bf16)
            V_aug = attn_pool.tile([P, NT, HPC, 72], fp8)
            OT = attn_pool.tile([P, EK, T], bf16)
            nc.vector.memset(V_aug[:, :, :, D : D + 1], 1.0)

            wp_cm = tc.tile_pool(name="wp", bufs=1)
            wp_pool = wp_cm.__enter__()
            wp_sb = wp_pool.tile([P, EK, C], bf16)
            nc.scalar.dma_start(
                wp_sb[:], wpT_d.ap().rearrange("(k p) o -> p k o", p=P))

            pt_cm = tc.tile_pool(name="ptp", bufs=4)
            pt_pool = pt_cm.__enter__()

            wearly_cm = tc.tile_pool(name="wearly", bufs=1)
            wearly = wearly_cm.__enter__()
            wq_sb = wearly.tile([P, CKD, 2, HPC * D], fp8)
            wk_sb = wearly.tile([P, CKD, 2, HPC * D], fp8)
            wv_sb = wearly.tile([P, CKD, 2, HPC * D], fp8)
            nc.gpsimd.dma_start(wv_sb[:], wv8_d.ap())
            nc.scalar.dma_start(wq_sb[:], wq8_d.ap())
            nc.scalar.dma_start(wk_sb[:], wk8_d.ap())
            xnT_cm = tc.tile_pool(name="p_xnT", bufs=1)
            p_xnT = xnT_cm.__enter__()
            xnTb_cm = tc.tile_pool(name="p_xnTb", bufs=1)
            p_xnTb = xnTb_cm.__enter__()
            # normalized x: bf16 transposed per-run buffer (f = 128ck + p),
            # cast to fp8 xnT8; DoubleRow k-tiles are chunk pairs (2j, 2j+1)
            xnT8 = p_xnT.tile([P, CK, T], fp8)


            xn2T_cm = tc.tile_pool(name="p_xn2T", bufs=1, side="right")
            p_xn2T = xn2T_cm.__enter__()
            xn2T = p_xn2T.tile([P, CK, T], bf16)

            def xnT8_dr(j, t0, n):
                # [P, 2, n] fp8 chunk-pair view for DoubleRow matmuls
                return xnT8[:, 2 * j : 2 * j + 2, t0 : t0 + n]

            def emit_ln1_dma(rr):
                xr = xpool.tile([P, 4, C], f32, tag="xres")
                nc.sync.dma_start(
                    xr[:],
                    x_d[rr * 512 : (rr + 1) * 512, :].rearrange(
                        "(t p) c -> p t c", p=P),
                )
                xnTb = p_xnTb.tile([P, CK, 512], bf16, tag="xnTb")
                return xr, xnTb

            def emit_ln1_chunk(rr, half, xr, xnTb):
                # LN1 for 2 tiles of run rr -> xnT8 (fp8, transposed)
                mvb = ln_pool.tile([P, 2, 2], f32, tag="ln1_mv")
                for i2 in range(2):
                    i = 2 * half + i2
                    xg = xr[:, i, :].rearrange("p (g f) -> p g f", f=512)
                    stats = ln_pool.tile([P, 2, 6], f32, tag="ln1_st")
                    for g in range(2):
                        nc.vector.bn_stats(out=stats[:, g, :], in_=xg[:, g, :])
                    nc.vector.bn_aggr(out=mvb[:, i2, :], in_=stats[:])
                rstd = rsqrt_newton(mvb[:, :, 1], 2, "ln1")
                for i2 in range(2):
                    i = 2 * half + i2
                    tt = 4 * rr + i
                    xn_bf = work.tile([P, C], bf16, tag="xn8")
                    nc.vector.tensor_scalar(
                        out=xn_bf[:], in0=xr[:, i, :],
                        scalar1=mvb[:, i2, 0:1], scalar2=rstd[:, i2 : i2 + 1],
                        op0=ALU.subtract, op1=ALU.mult,
                    )
                    nc.sync.dma_start_transpose(
                        xnTb[:, :, i * P : (i + 1) * P], xn_bf[:])
                    nc.vector.tensor_copy(
                        out=xnT8[:, :, tt * P : (tt + 1) * P],
                        in_=xnTb[:, :, i * P : (i + 1) * P],
                    )

            def emit_ln1_run(rr):
                xr, xnTb = emit_ln1_dma(rr)
                emit_ln1_chunk(rr, 0, xr, xnTb)
                emit_ln1_chunk(rr, 1, xr, xnTb)

            def emit_v_tile(tt):
                ps = ppool.tile([P, 512], f32, tag="mm")
                for ck in range(CKD):
                    nc.tensor.matmul(
                        ps[:],
                        xnT8_dr(ck, tt * P, P),
                        wv_sb[:, ck, :, :],
                        start=(ck == 0), stop=(ck == CKD - 1),
                        perf_mode=DR,
                    )
                nc.vector.tensor_copy(
                    out=V_aug[:, tt, :, 0:D],
                    in_=ps[:].rearrange("p (h e) -> p h e", h=HPC),
                )

            def emit_qk_tile(r, ot):
                # ot in 0..8: 0-3 Q tiles, 4-7 K tiles (natural 128-col chunks)
                w_sb = wq_sb if ot < 4 else wk_sb
                dst = QT if ot < 4 else KT
                ti = ot % 4
                ps = ppool.tile([P, 512], f32, tag="mm")
                for ck in range(CKD):
                    nc.tensor.matmul(
                        ps[:],
                        w_sb[:, ck, :, ti * P : (ti + 1) * P],
                        xnT8_dr(ck, r * 512, 512),
                        start=(ck == 0), stop=(ck == CKD - 1),
                        perf_mode=DR,
                    )
                nc.vector.tensor_scalar(
                    out=dst[:, ti, r * 512 : (r + 1) * 512],
                    in0=ps[:], scalar1=bqk_sb[:, ot : ot + 1], scalar2=None,
                    op0=ALU.add,
                )

            def emit_x2_run(rr):
                # x2 = x + cc (attn partial sum incl b_proj); LN2; transpose
                mvb = ln_pool.tile([P, 4, 2], f32, tag="ln2_mv")
                xr = xpool.tile([P, 4, C], f32, tag="xres")
                dslice = x_d[rr * 512 : (rr + 1) * 512, :].rearrange(
                    "(t p) c -> p t c", p=P)
                nc.sync.dma_start(xr[:], dslice)
                att_sb = xpool.tile([P, 4, C], bf16, tag="attres")
                nc.sync.dma_start(
                    att_sb[:],
                    cc_out_d[rr * 512 : (rr + 1) * 512, :].rearrange(
                        "(t p) c -> p t c", p=P),
                )
                nc.vector.tensor_add(out=xr[:], in0=xr[:], in1=att_sb[:])
                nc.sync.dma_start(
                    out_x2_d[rr * 512 : (rr + 1) * 512, :].rearrange(
                        "(t p) c -> p t c", p=P),
                    xr[:],
                )
                x2s = []
                for i in range(4):
                    x_sb = xr[:, i, :]
                    xg = x_sb.rearrange("p (g f) -> p g f", f=512)
                    stats = ln_pool.tile([P, 2, 6], f32, tag="ln2_st")
                    for g in range(2):
                        nc.vector.bn_stats(out=stats[:, g, :], in_=xg[:, g, :])
                    nc.vector.bn_aggr(out=mvb[:, i, :], in_=stats[:])
                    x2s.append(x_sb)
                rstd = rsqrt_newton(mvb[:, :, 1], 4, "ln2")
                for i in range(4):
                    tt = 4 * rr + i
                    xn2_bf = work.tile([P, C], bf16, tag="xn2bf")
                    nc.vector.tensor_scalar(
                        out=xn2_bf[:], in0=x2s[i],
                        scalar1=mvb[:, i, 0:1], scalar2=rstd[:, i : i + 1],
                        op0=ALU.subtract, op1=ALU.mult,
                    )
                    nc.sync.dma_start_transpose(
                        xn2T[:, :, tt * P : (tt + 1) * P], xn2_bf[:])

            # ======== fused pipeline over the 4 token runs ========
            for r in range(NR):
                if r == 0:
                    xr0, xnTb0 = emit_ln1_dma(0)
                    emit_ln1_chunk(0, 0, xr0, xnTb0)
                    emit_v_tile(0)
                    emit_v_tile(1)
                    emit_ln1_chunk(0, 1, xr0, xnTb0)
                    emit_v_tile(2)
                    emit_v_tile(3)
                    for ot in range(8):
                        emit_qk_tile(0, ot)
                fillers = []
                if r < NR - 1:
                    emit_ln1_run(r + 1)
                    for tt in range(4 * (r + 1), 4 * (r + 1) + 4):
                        fillers.append(lambda tt=tt: emit_v_tile(tt))
                    for ot in range(8):
                        fillers.append(lambda ot=ot: emit_qk_tile(r + 1, ot))

                # --- attention: heads processed in interleaved pairs so the
                # tensor engine always has independent work while exp runs ---
                ns = 4 * r + 4
                npairs = ns // 2
                pending_mul = []

                def emit_sc(h, st):
                    hp = (h % 2) * D
                    hc = h // 2
                    sc = scpool.tile([P, 512], f32, tag="sc")
                    nc.tensor.matmul(
                        sc[:],
                        KT[hp : hp + D, hc, st * P : (st + 1) * P],
                        QT[hp : hp + D, hc, r * 512 : (r + 1) * 512],
                        start=True, stop=True,
                    )
                    return sc

                def emit_exp(st, sc, PT):
                    j = st - 4 * r
                    off = (st % 2) * 512
                    if j < 0:
                        nc.scalar.activation(
                            out=PT[:, off : off + 512], in_=sc[:],
                            func=AF.Exp, scale=0.125)
                    else:
                        nc.vector.tensor_add(
                            out=sc[:, j * P : (j + 1) * P],
                            in0=sc[:, j * P : (j + 1) * P],
                            in1=negmask_sb[:],
                        )
                        nc.scalar.activation(
                            out=PT[:, off + j * P : off + 512],
                            in_=sc[:, j * P : 512],
                            func=AF.Exp, scale=0.125)
                        if j > 0:
                            nc.gpsimd.memset(PT[:, off : off + j * P], 0.0)

                carry = None
                for h0 in range(0, HPC, 2):
                    heads = (h0, h0 + 1)
                    po_a = pvpool.tile([P, 512], f32, tag="pv0")
                    po_b = pvpool.tile([P, 512], f32, tag="pv1")
                    pos = [po_a, po_b]
                    PTs = [None, None]
                    buf = {}
                    if carry is not None:
                        buf[(0, 0)], buf[(1, 0)] = carry
                        carry = None
                    else:
                        for x in range(2):
                            buf[(x, 0)] = emit_sc(heads[x], 0)
                    for st in range(ns):
                        for x in range(2):
                            if st + 1 < ns:
                                buf[(x, st + 1)] = emit_sc(heads[x], st + 1)
                        if st == ns - 1 and h0 + 2 < HPC:
                            # next pair's first scores overlap our denominators
                            carry = (emit_sc(h0 + 2, 0), emit_sc(h0 + 3, 0))
                        if st == 0:
                            while pending_mul:
                                pending_mul.pop(0)()
                        for x in range(2):
                            if st % 2 == 0:
                                PT_new = pt_pool.tile([P, 1024], fp8, tag="PT")
                                PTs[x] = PT_new
                            emit_exp(st, buf.pop((x, st)), PTs[x])
                        if st % 2 == 1:
                            pi = st // 2
                            for x in range(2):
                                nc.tensor.matmul(
                                    pos[x][: D + 1, :],
                                    V_aug[:, 2 * pi : 2 * pi + 2,
                                          heads[x], 0 : D + 1],
                                    PTs[x][:].rearrange(
                                        "p (two n) -> p two n", two=2),
                                    start=(pi == 0), stop=(pi == npairs - 1),
                                    perf_mode=DR,
                                )
                    # denominators for both heads (multiplies deferred into
                    # the next pair so they never block its exp chain)
                    for x in range(2):
                        h = heads[x]
                        hp = (h % 2) * D
                        hc = h // 2
                        po = pos[x]
                        dsum = small.tile([1, 512], f32, tag="dsum")
                        nc.scalar.activation(
                            out=dsum[:], in_=po[D : D + 1, :], func=AF.Identity)
                        rec = small.tile([1, 512], f32, tag="rec")
                        nc.vector.reciprocal_approx_fast(out=rec[:], in_=dsum[:])
                        den = denp.tile([D, 512], f32, tag="den")
                        nc.gpsimd.partition_broadcast(den[:], rec[:])

                        def ot_mul(po=po, den=den, hp=hp, hc=hc):
                            nc.vector.tensor_mul(
                                out=OT[hp : hp + D, hc, r * 512 : (r + 1) * 512],
                                in0=po[0:D, :],
                                in1=den[:],
                            )
                        pending_mul.append(ot_mul)

                    # drain filler tensor work (next run's V/QK)
                    npair_left = (HPC - h0) // 2
                    take = (len(fillers) + npair_left - 1) // npair_left
                    for _ in range(take):
                        fillers.pop(0)()

                while pending_mul:
                    pending_mul.pop(0)()

                # --- c_proj partial (+bproj/2) + AllReduce chunk ---
                for tt in range(4 * r, 4 * r + 4):
                    cc_sb = work.tile([P, C], bf16, tag="ccbuf")
                    for half in range(2):
                        ps = ppool.tile([P, 512], f32, tag="mm")
                        for ek in range(EK):
                            nc.tensor.matmul(
                                ps[:],
                                OT[:, ek, tt * P : (tt + 1) * P],
                                wp_sb[:, ek, half * 512 : (half + 1) * 512],
                                start=(ek == 0), stop=(ek == EK - 1),
                            )
                        nc.vector.tensor_add(
                            out=cc_sb[:, half * 512 : (half + 1) * 512],
                            in0=ps[:],
                            in1=bproj_sb[:, half * 512 : (half + 1) * 512],
                        )
                    nc.sync.dma_start(
                        cc_in_d[tt * P : (tt + 1) * P, :], cc_sb[:])

                nc.gpsimd.collective_compute(
                    "AllReduce",
                    ALU.add,
                    replica_groups=[[0, 1], [2, 3], [4, 5], [6, 7]],
                    ins=[cc_in_d[r * 512 : (r + 1) * 512, :].opt()],
                    outs=[cc_out_d[r * 512 : (r + 1) * 512, :].opt()],
                )
                if r == NR - 1:
                    emit_x2_run(0)
                    emit_x2_run(1)
                if r == NR - 2:
                    # run-3 QKV work is already emitted; free its inputs and
                    # prefetch the first fc weight chunks during run 3
                    xnTb_cm.__exit__(None, None, None)
                    xnT_cm.__exit__(None, None, None)
                    wearly_cm.__exit__(None, None, None)
                    wfe_cm = tc.tile_pool(name="wfearly", bufs=1, side="right")
                    wfe = wfe_cm.__enter__()
                    wfA = wfe.tile([P, 5, FPC], bf16)
                    wfT_r = wfT_d.ap().rearrange("(k p) o -> p k o", p=P)
                    for ck in range(5):
                        eng = nc.scalar if ck % 2 == 0 else nc.gpsimd
                        eng.dma_start(wfA[:, ck, :], wfT_r[:, ck, :])

            # release attention-phase SBUF before the MLP phase
            pt_cm.__exit__(None, None, None)
            wp_cm.__exit__(None, None, None)
            attn_cm.__exit__(None, None, None)

            with tc.tile_pool(name="wlate", bufs=1, side="right") as wlate, \
                 tc.tile_pool(name="p_hT", bufs=1, side="right") as p_hT:
                wfB = wlate.tile([P, CK - 5, FPC], bf16)
                wfT_r = wfT_d.ap().rearrange("(k p) o -> p k o", p=P)
                for ck in range(5, CK):
                    eng = nc.scalar if ck % 2 == 0 else nc.gpsimd
                    eng.dma_start(wfB[:, ck - 5, :], wfT_r[:, ck, :])
                wm_sb = wlate.tile([P, FK, C], bf16)
                wmT_r = wmT_d.ap().rearrange("(k p) o -> p k o", p=P)
                for fk in range(0, FK, 4):
                    eng = nc.scalar if (fk // 4) % 2 == 0 else nc.gpsimd
                    eng.dma_start(
                        wm_sb[:, fk : fk + 4, :], wmT_r[:, fk : fk + 4, :])

                # ======== MLP in 4 token quarters ========
                for tq in range(4):
                    if tq in (0, 1):
                        emit_x2_run(tq + 2)
                    t0 = tq * 512
                    hT = p_hT.tile([P, FK, 512], bf16, tag="hT")
                    for ft in range(FK):
                        ps = ppool.tile([P, 512], f32, tag="mm")
                        for ck in range(CK):
                            wsl = (wfA[:, ck, ft * P : (ft + 1) * P]
                                   if ck < 5 else
                                   wfB[:, ck - 5, ft * P : (ft + 1) * P])
                            nc.tensor.matmul(
                                ps[:],
                                wsl,
                                xn2T[:, ck, t0 : t0 + 512],
                                start=(ck == 0), stop=(ck == CK - 1),
                            )
                        nc.scalar.activation(
                            out=hT[:, ft, :], in_=ps[:],
                            func=AF.Gelu_apprx_tanh,
                            bias=bfc_sb[:, ft : ft + 1], scale=1.0,
                        )
                    for tl in range(4):
                        out_sb = work.tile([P, C], f32, tag="f32buf")
                        for half in range(2):
                            ps = ppool.tile([P, 512], f32, tag="mm")
                            for fk in range(FK):
                                nc.tensor.matmul(
                                    ps[:],
                                    hT[:, fk, tl * P : (tl + 1) * P],
                                    wm_sb[:, fk, half * 512 : (half + 1) * 512],
                                    start=(fk == 0), stop=(fk == FK - 1),
                                )
                            nc.vector.tensor_copy(
                                out=out_sb[:, half * 512 : (half + 1) * 512],
                                in_=ps[:],
                            )
                        nc.gpsimd.dma_start(
                            out_mlp_d[t0 + tl * P : t0 + (tl + 1) * P, :],
                            out_sb[:],
                        )

            wfe_cm.__exit__(None, None, None)
            xn2T_cm.__exit__(None, None, None)

    nc.finalize()
    return nc


def _prep_inputs(x, w_attn, b_attn, w_proj, b_proj, w_fc, b_fc, w_mlp_proj):
    bf = ml_dtypes.bfloat16
    f8 = ml_dtypes.float8_e4m3
    negmask = np.where(
        np.triu(np.ones((P, P), dtype=np.float32)) > 0, 0.0, -1e5
    ).astype(np.float32)

    # lhsT column permutation for Q/K tiles: tile = 2g+ktd, col m ->
    # row 64*(4g + m//32) + 32*ktd + m%32 of the local weight slice
    tiles = np.arange(4)
    m = np.arange(P)
    g = tiles // 2
    ktd = tiles % 2
    rows = (64 * (4 * g[:, None] + m[None, :] // 32)
            + 32 * ktd[:, None] + m[None, :] % 32)  # [4, 128]
    qk_rows = rows.reshape(-1)  # [512]

    def dr_pack(wl, permute):
        # wl [512 out, 1024 feat] -> [128p, 4ck, 2kt, 512 out] fp8
        # feature f = 256*ck + 2*p + kt
        if permute:
            wl = wl[qk_rows, :]
        w4 = wl.reshape(512, CKD, 2, P)          # [out, j, kt, p]
        return np.ascontiguousarray(w4.transpose(3, 1, 2, 0)).astype(f8)

    in_maps = []
    for core in range(8):
        b, s = divmod(core, 2)
        wq = w_attn[s * 512 : (s + 1) * 512, :]
        wk = w_attn[C + s * 512 : C + (s + 1) * 512, :]
        wv = w_attn[2 * C + s * 512 : 2 * C + (s + 1) * 512, :]
        bq = b_attn[s * 512 : (s + 1) * 512]
        bk = b_attn[C + s * 512 : C + (s + 1) * 512]
        bv = b_attn[2 * C + s * 512 : 2 * C + (s + 1) * 512]
        bqk = np.concatenate(
            [bq.reshape(EK, P).T, bk.reshape(EK, P).T], axis=1
        ).astype(np.float32)  # [128, 8] (4 Q tiles, 4 K tiles)
        wp = np.ascontiguousarray(w_proj[:, s * 512 : (s + 1) * 512].T).astype(bf)
        wf = np.ascontiguousarray(w_fc[s * FPC : (s + 1) * FPC, :].T).astype(bf)
        bfc = np.ascontiguousarray(
            b_fc[s * FPC : (s + 1) * FPC].reshape(FK, P).T).astype(np.float32)
        wm = np.ascontiguousarray(
            w_mlp_proj[:, s * FPC : (s + 1) * FPC].T).astype(bf)
        in_maps.append(
            {
                "x": np.ascontiguousarray(x[b]),
                "wq8": dr_pack(wq, False),
                "wk8": dr_pack(wk, False),
                "wv8": dr_pack(wv, False),
                "wpT": wp, "wfT": wf, "wmT": wm,
                "bqk": np.ascontiguousarray(bqk),
                "bv": np.ascontiguousarray(bv).astype(np.float32),
                "bprojh": ((b_proj + w_proj @ b_attn[2 * C : 3 * C]) if s == 0
                           else np.zeros(C, np.float32)).astype(bf),
                "bfc": bfc, "negmask": negmask,
            }
        )
    return in_maps


def run(x, w_attn, b_attn, w_proj, b_proj, w_fc, b_fc, w_mlp_proj, b_mlp_proj,
        trace=False):
    from concourse.bass_utils import run_bass_kernel_spmd

    if "nc" not in _CACHED:
        _CACHED["nc"] = _build_nc()
    nc = _CACHED["nc"]
    in_maps = _prep_inputs(
        x, w_attn, b_attn, w_proj, b_proj, w_fc, b_fc, w_mlp_proj
    )
    res = run_bass_kernel_spmd(
        nc, in_maps, core_ids=list(range(8)), trace=trace,
        trace_cores=list(range(8)) if trace else None,
    )
    out = np.empty((B, T, C), dtype=np.float32)
    for b in range(B):
        a = res.results[2 * b]
        c2 = res.results[2 * b + 1]
        out[b] = a["out_x2"] + a["out_mlp"] + c2["out_mlp"] + b_mlp_proj[None, :]
    return out, res


def kernel(x, w_attn, b_attn, w_proj, b_proj, w_fc, b_fc, w_mlp_proj, b_mlp_proj):
    out, _ = run(
        np.asarray(x, dtype=np.float32),
        np.asarray(w_attn, dtype=np.float32),
        np.asarray(b_attn, dtype=np.float32),
        np.asarray(w_proj, dtype=np.float32),
        np.asarray(b_proj, dtype=np.float32),
        np.asarray(w_fc, dtype=np.float32),
        np.asarray(b_fc, dtype=np.float32),
        np.asarray(w_mlp_proj, dtype=np.float32),
        np.asarray(b_mlp_proj, dtype=np.float32),
    )
    return out
